# revision 1
# baseline (speedup 1.0000x reference)
import sys

sys.path.insert(0, "/opt/trn_rl_repo")

import math

import numpy as np

import concourse.bass as bass
import concourse.mybir as mybir
import concourse.tile as tile
from concourse import bacc
from concourse.bass_utils import run_bass_kernel_spmd
from concourse.masks import make_identity

F32 = mybir.dt.float32
F32R = mybir.dt.float32r
IDENT = mybir.ActivationFunctionType.Identity
EXPF = mybir.ActivationFunctionType.Exp

B, S, D = 8, 1024, 1024
N_H = 16
REL_K = 16
d_k = D // N_H  # 64
N_CORES = 8
MASKVAL = -1e30
NBUF = 6

_CACHE = {}
TRACE = False


def build_module():
    nc = bacc.Bacc("TRN2", detect_race_conditions=False, num_swdge_queues=4)

    xT = nc.dram_tensor("xT", [D, S], F32R, kind="ExternalInput")
    Wqk = nc.dram_tensor("Wqk", [D, 2 * D], F32R, kind="ExternalInput")
    Wv = nc.dram_tensor("Wv", [D, D], F32R, kind="ExternalInput")
    Wp = nc.dram_tensor("Wp", [D, D], F32R, kind="ExternalInput")
    bqk = nc.dram_tensor("bqk", [128, 16], F32, kind="ExternalInput")
    bvp = nc.dram_tensor("bvp", [1, D], F32R, kind="ExternalInput")
    bp = nc.dram_tensor("bp", [1, D], F32R, kind="ExternalInput")
    dlut = nc.dram_tensor("dlut", [d_k, 16], F32R, kind="ExternalInput")
    dlv = nc.dram_tensor("dlv", [16, d_k], F32, kind="ExternalInput")
    selm = nc.dram_tensor("selm", [16, 1024], F32R, kind="ExternalInput")
    zb2init = nc.dram_tensor("zb2init", [128, 160], F32, kind="ExternalInput")
    OUT = nc.dram_tensor("OUT", [S, D], F32, kind="ExternalOutput")

    zb2 = [nc.dram_tensor(f"zb2_{k}", [128, 160], F32) for k in range(8)]
    zdp = [nc.dram_tensor(f"zdp_{k}", [16, 1040], F32) for k in range(2)]
    zb1 = [nc.dram_tensor(f"zb1_{k}", [16, 160], F32) for k in range(NBUF)]
    ewd = [nc.dram_tensor(f"ewd_{k}", [128, 144], F32) for k in range(NBUF)]

    with tile.TileContext(nc) as tc:
        with (
            tc.tile_pool(name="persist", bufs=1) as pers,
            tc.tile_pool(name="small", bufs=5) as small,
            tc.tile_pool(name="ps_big", bufs=1, space="PSUM") as ps_big,
        ):
            # ---- constants ----
            ident = pers.tile([128, 128], F32)
            make_identity(nc, ident[:])
            identR = pers.tile([128, 128], F32R)
            nc.vector.tensor_copy(identR[:], ident[:])
            dlut_sb = pers.tile([128, 16], F32R)
            nc.sync.dma_start(out=dlut_sb[0:64, :], in_=dlut[:])
            nc.sync.dma_start(out=dlut_sb[64:128, :], in_=dlut[:])
            dlv_sb = pers.tile([16, d_k], F32R)
            nc.gpsimd.dma_start(out=dlv_sb[:], in_=dlv[:])
            selm_sb = pers.tile([16, 1024], F32R)
            nc.sync.dma_start(out=selm_sb[:], in_=selm[:])
            bqk_sb = pers.tile([128, 16], F32)
            nc.gpsimd.dma_start(out=bqk_sb[:], in_=bqk[:])
            bvp_sb = pers.tile([1, D], F32R)
            nc.sync.dma_start(out=bvp_sb[:], in_=bvp[:])
            bp_sb = pers.tile([1, D], F32R)
            nc.sync.dma_start(out=bp_sb[:], in_=bp[:])
            ones1f = pers.tile([1, 128], F32)
            nc.vector.memset(ones1f[:], 1.0)
            ones1 = pers.tile([1, 128], F32R)
            nc.vector.tensor_copy(ones1[:], ones1f[:])
            onescol_f = pers.tile([128, 16], F32)
            nc.vector.memset(onescol_f[:], 1.0)

            z16 = pers.tile([16, 160], F32)
            nc.vector.memset(z16[:], 0.0)
            zi_sb = pers.tile([128, 160], F32)
            nc.gpsimd.dma_start(out=zi_sb[:], in_=zb2init[:])
            for k in range(NBUF):
                nc.sync.dma_start(out=zb1[k][:], in_=z16[:])
            for k in range(8):
                nc.sync.dma_start(out=zb2[k][:], in_=zi_sb[:])

            # ---- load xT (f32r) ----
            xT_sb = []
            for d in range(8):
                t = pers.tile([128, S], F32R, tag=f"xT{d}")
                nc.sync.dma_start(out=t[:], in_=xT[128 * d:128 * (d + 1), :])
                xT_sb.append(t)

            # ---- v projection -> vhat_sb (65-stride layout + ones cols) ----
            vhat_sb = [pers.tile([128, 16 * 65], F32R, name=f"vh{jt}", tag=f"vh{jt}")
                       for jt in range(8)]
            with tc.tile_pool(name="wv", bufs=1) as wvp:
                Wv_sb = []
                for d in range(8):
                    t = wvp.tile([128, D], F32R, tag=f"wv{d}")
                    nc.sync.dma_start(out=t[:], in_=Wv[128 * d:128 * (d + 1), :])
                    Wv_sb.append(t)
                for tt in range(8):
                    vt = vhat_sb[tt]
                    ones_ap = bass.AP(tensor=vt[:].tensor, offset=64,
                                      ap=[[16 * 65, 128], [65, 16]])
                    nc.vector.tensor_copy(ones_ap, onescol_f[:])
                    for fc in range(2):
                        ps = ps_big.tile([128, 512], F32, tag="pbig")
                        for d in range(8):
                            nc.tensor.matmul(
                                ps[:],
                                xT_sb[d][:, 128 * tt:128 * (tt + 1)],
                                Wv_sb[d][:, 512 * fc:512 * (fc + 1)],
                                start=(d == 0), stop=False,
                            )
                        nc.tensor.matmul(
                            ps[:],
                            ones1[:],
                            bvp_sb[:, 512 * fc:512 * (fc + 1)],
                            start=False, stop=True,
                        )
                        srcA = bass.AP(tensor=ps[:].tensor,
                                       offset=ps[:].offset,
                                       ap=[[512, 128], [64, 8], [1, 64]])
                        dst = bass.AP(tensor=vt[:].tensor, offset=65 * 8 * fc,
                                      ap=[[16 * 65, 128], [65, 8], [1, 64]])
                        nc.scalar.copy(dst, srcA)

            pair_sb = [pers.tile([128, S], F32R, name=f"pair{hp}", tag=f"pair{hp}")
                       for hp in range(8)]
            denoms = pers.tile([16, S], F32)

            # ---- attention ----
            with (
                tc.tile_pool(name="wqk", bufs=12) as wqkp,
                tc.tile_pool(name="qk", bufs=2) as qkp,
                tc.tile_pool(name="bandp", bufs=2) as bandp,
                tc.tile_pool(name="outtp", bufs=2) as outtp,
                tc.tile_pool(name="eskp", bufs=2) as eskp,
                tc.tile_pool(name="dtp", bufs=1) as dtp,
                tc.tile_pool(name="att", bufs=4) as attp,
                tc.tile_pool(name="dpp", bufs=2) as dpp,
                tc.tile_pool(name="ps_s", bufs=4, space="PSUM") as ps_s,
                tc.tile_pool(name="ps_out", bufs=1, space="PSUM") as ps_out,
                tc.tile_pool(name="ps_sm", bufs=1, space="PSUM") as ps_sm,
            ):
                rot = 0

                def emit_qkproj(hp2):
                    qk_pair = []
                    for sec, ft in ((0, hp2), (1, 8 + hp2)):
                        ws = []
                        for d in range(8):
                            w = wqkp.tile([128, 128], F32R, tag="wqk")
                            nc.gpsimd.dma_start(
                                out=w[:],
                                in_=Wqk[128 * d:128 * (d + 1), 128 * ft:128 * (ft + 1)])
                            ws.append(w)
                        dstt = qkp.tile([128, S], F32R, tag=f"qk{sec}")
                        for tch in range(2):
                            ps = ps_big.tile([128, 512], F32, tag="pbig")
                            for d in range(8):
                                nc.tensor.matmul(
                                    ps[:],
                                    ws[d][:],
                                    xT_sb[d][:, 512 * tch:512 * (tch + 1)],
                                    start=(d == 0), stop=(d == 7),
                                )
                            nc.scalar.activation(dstt[:, 512 * tch:512 * (tch + 1)],
                                                 ps[:], IDENT,
                                                 bias=bqk_sb[:, ft:ft + 1], scale=1.0)
                        qk_pair.append(dstt)
                    return qk_pair

                next_qk = emit_qkproj(0)
                for hp in range(8):
                    qT_pair, kT_pair = next_qk

                    for hh in range(2):
                        h = 2 * hp + hh
                        po = hh * 64
                        qT = qT_pair[po:po + 64, :]
                        kT = kT_pair[po:po + 64, :]

                        # dp matmuls -> dpT [16, 1040]
                        dpT = dpp.tile([16, 1040], F32, tag="dpT")
                        nc.vector.memset(dpT[:, 1024:1040], 0.0)
                        for c in range(2):
                            psdp = ps_s.tile([128, 512], F32, tag="pss")
                            nc.tensor.matmul(psdp[0:16, :],
                                             dlut_sb[po:po + 64, :],
                                             qT[:, 512 * c:512 * (c + 1)],
                                             start=True, stop=True)
                            nc.vector.tensor_copy(dpT[:, 512 * c:512 * (c + 1)],
                                                  psdp[0:16, :])
                        dpSh = dpp.tile([16, 1040], F32, tag="dpSh")
                        zdp_i = h % 2
                        nc.scalar.dma_start(out=zdp[zdp_i][:], in_=dpT[:])
                        srcSh = bass.AP(tensor=zdp[zdp_i][:].tensor, offset=0,
                                        ap=[[1041, 16], [1, 1024]])
                        nc.scalar.dma_start(out=dpSh[:, 0:1024], in_=srcSh)

                        band_tiles = []
                        for jt in range(8):
                            j0 = 128 * jt
                            psd = ps_sm.tile([128, 128], F32, tag="pstr")
                            nc.tensor.transpose(psd[0:128, 0:16],
                                                dpSh[:, j0:j0 + 128],
                                                ident[0:16, 0:16])
                            dpS = small.tile([128, 16], F32, tag="dpS")
                            nc.vector.tensor_copy(dpS[:], psd[0:128, 0:16])
                            dstW = bass.AP(tensor=zb2[jt][:].tensor, offset=0,
                                           ap=[[161, 128], [1, 16]])
                            nc.scalar.dma_start(out=dstW, in_=dpS[:])
                            band = bandp.tile([128, 160], F32, name=f"band{jt}",
                                              tag=f"band{jt}")
                            nc.gpsimd.dma_start(out=band[:], in_=zb2[jt][:])
                            band_tiles.append(band)

                        pso = ps_out.tile([65, 1024], F32, tag="pso")
                        esk_tiles = []

                        for jt in range(8):
                            j0 = 128 * jt
                            wdiag = min(512, S - j0)
                            win = min(144, S - j0)
                            pss0 = ps_s.tile([128, 512], F32, tag="pss")
                            nc.tensor.matmul(pss0[:, 0:wdiag],
                                             kT[:, j0:j0 + 128],
                                             qT[:, j0:j0 + wdiag],
                                             start=True, stop=True)
                            band = band_tiles[jt]
                            expT = attp.tile([128, 1024], F32R, tag="expT")
                            sS = small.tile([128, 144], F32, tag="sS")
                            nc.vector.tensor_add(sS[:, 0:win], pss0[:, 0:win],
                                                 band[:, 0:win])
                            nc.scalar.activation(expT[:, 0:win], sS[:, 0:win], EXPF)
                            zb_i = rot % NBUF
                            rot += 1
                            ew = small.tile([128, 144], F32, tag="ew")
                            if win < 144:
                                nc.vector.memset(ew[:, win:144], 0.0)
                            nc.vector.tensor_copy(ew[:, 0:win], expT[:, 0:win])
                            nc.sync.dma_start(out=ewd[zb_i][:], in_=ew[:])
                            esk = eskp.tile([128, 16], F32, name=f"esk{jt}",
                                            tag=f"esk{jt}")
                            srcR = bass.AP(tensor=ewd[zb_i][:].tensor, offset=0,
                                           ap=[[145, 128], [1, 16]])
                            nc.sync.dma_start(out=esk[:], in_=srcR)
                            esk_tiles.append((esk, zb_i))
                            if wdiag > win:
                                nc.scalar.activation(expT[:, win:wdiag],
                                                     pss0[:, win:wdiag], EXPF)
                            if S - j0 > 512:
                                w1 = S - j0 - 512
                                pss1 = ps_s.tile([128, 512], F32, tag="pss")
                                nc.tensor.matmul(pss1[:, 0:w1],
                                                 kT[:, j0:j0 + 128],
                                                 qT[:, j0 + 512:S],
                                                 start=True, stop=True)
                                nc.scalar.activation(expT[:, 512:512 + w1],
                                                     pss1[:, 0:w1], EXPF)
                            lhs = vhat_sb[jt][:, 65 * h:65 * h + 65]
                            segs = ([(j0, 512), (512, 1024)] if j0 < 512
                                    else [(j0, 1024)])
                            for (a, b2) in segs:
                                nc.tensor.matmul(pso[:, a:b2], lhs,
                                                 expT[:, a - j0:b2 - j0],
                                                 start=(jt == 0), stop=False,
                                                 skip_group_check=True)

                        if hh == 1 and hp < 7:
                            next_qk = emit_qkproj(hp + 1)
                        # phase 2: transposes + shear-out + dt reads
                        dt_tiles = []
                        for jt in range(8):
                            esk, zb_i = esk_tiles[jt]
                            pst = ps_sm.tile([128, 128], F32, tag="pstr")
                            nc.tensor.transpose(pst[0:16, 0:128], esk[:], ident[:])
                            t1 = small.tile([16, 128], F32, tag="t1")
                            nc.vector.tensor_copy(t1[:], pst[0:16, 0:128])
                            dstZ = bass.AP(tensor=zb1[zb_i][:].tensor, offset=0,
                                           ap=[[161, 16], [1, 128]])
                            nc.scalar.dma_start(out=dstZ, in_=t1[:])
                            dt_sb = dtp.tile([16, 160], F32R, name=f"dt{jt}",
                                             tag=f"dt{jt}")
                            nc.gpsimd.dma_start(out=dt_sb[:], in_=zb1[zb_i][:])
                            dt_tiles.append(dt_sb)
                        esk_tiles.clear()

                        # phase 3: DT matmuls
                        for jt in range(8):
                            j0 = 128 * jt
                            win = min(144, S - j0)
                            dt_sb = dt_tiles[jt]
                            a0, b0 = j0, j0 + win
                            dsegs = ([(a0, 512), (512, b0)] if (a0 < 512 < b0)
                                     else [(a0, b0)])
                            for (a, b2) in dsegs:
                                nc.tensor.matmul(pso[0:64, a:b2], dlv_sb[:],
                                                 dt_sb[:, a - j0:b2 - j0],
                                                 start=False,
                                                 stop=(jt == 7 and (a, b2) == dsegs[-1]),
                                                 skip_group_check=True)

                        # evict head result via SBUF (DMA moves partitions)
                        outT_sb = outtp.tile([65, 1024], F32R, tag="outT")
                        nc.scalar.copy(outT_sb[:], pso[:])
                        nc.sync.dma_start(out=pair_sb[hp][po:po + 64, :],
                                          in_=outT_sb[0:64, :])
                        nc.sync.dma_start(out=denoms[h:h + 1, :].bitcast(F32R),
                                          in_=outT_sb[64:65, :])


            # ---- normalize ----
            recip = pers.tile([16, S], F32R)
            with nc.allow_low_precision(reason="f32r rounding for matmul broadcast"):
                nc.vector.reciprocal(recip[:], denoms[:])
            with tc.tile_pool(name="ps_n", bufs=2, space="PSUM") as ps_n:
                for hp in range(8):
                    psb = ps_n.tile([128, 1024], F32, tag="psb")
                    for c in range(2):
                        nc.tensor.matmul(psb[:, 512 * c:512 * (c + 1)],
                                         selm_sb[:, 128 * hp:128 * (hp + 1)],
                                         recip[:, 512 * c:512 * (c + 1)],
                                         start=True, stop=True)
                    nc.vector.tensor_mul(pair_sb[hp][:], pair_sb[hp][:], psb[:])

            # ---- final projection ----
            with (
                tc.tile_pool(name="wp", bufs=1) as wpp,
                tc.tile_pool(name="ps_p", bufs=2, space="PSUM") as ps_p,
                tc.tile_pool(name="outp", bufs=2) as outp,
            ):
                Wp_sb = []
                for d in range(8):
                    t = wpp.tile([128, D], F32R, tag=f"wp{d}")
                    nc.sync.dma_start(out=t[:], in_=Wp[128 * d:128 * (d + 1), :])
                    Wp_sb.append(t)
                for tt in range(8):
                    ps = ps_p.tile([128, 1024], F32, tag="psp")
                    for fc in range(2):
                        for d in range(8):
                            nc.tensor.matmul(
                                ps[:, 512 * fc:512 * (fc + 1)],
                                pair_sb[d][:, 128 * tt:128 * (tt + 1)],
                                Wp_sb[d][:, 512 * fc:512 * (fc + 1)],
                                start=(d == 0), stop=False,
                            )
                            pass
                        nc.tensor.matmul(
                            ps[:, 512 * fc:512 * (fc + 1)],
                            ones1[:],
                            bp_sb[:, 512 * fc:512 * (fc + 1)],
                            start=False, stop=True,
                        )
                    ot = outp.tile([128, 1024], F32, tag="ot")
                    nc.vector.tensor_copy(ot[:], ps[:])
                    nc.sync.dma_start(out=OUT[128 * tt:128 * (tt + 1), :], in_=ot[:])

    nc.compile()
    return nc


def _host_prep(W_attn, b_attn, W_proj, b_proj, lut_k, lut_v):
    scale = 1.0 / math.sqrt(d_k)
    Wqk = np.concatenate([W_attn[:, :D], W_attn[:, D:2 * D] * scale], axis=1)
    bq = b_attn[:D]
    bk = b_attn[D:2 * D] * scale
    bqk_h = np.stack([np.concatenate([bq, bk])[128 * ft:128 * (ft + 1)]
                      for ft in range(16)], axis=1).astype(np.float32)
    bvp_h = (b_attn[2 * D:3 * D] + np.tile(lut_v[0], N_H)).reshape(1, D)
    dlut_h = np.stack([(lut_k[16 - u] - lut_k[0]) * scale for u in range(16)],
                      axis=1).astype(np.float32)
    dlv_h = np.stack([lut_v[16 - u] - lut_v[0] for u in range(16)],
                     axis=0).astype(np.float32)
    selm_h = np.zeros((16, 1024), np.float32)
    for hp in range(8):
        for p in range(128):
            selm_h[2 * hp + p // 64, 128 * hp + p] = 1.0
    zb2_h = np.where(np.arange(160)[None, :] < np.arange(128)[:, None],
                     np.float32(MASKVAL), np.float32(0.0)).astype(np.float32)
    return {
        "Wqk": np.ascontiguousarray(Wqk, np.float32),
        "Wv": np.ascontiguousarray(W_attn[:, 2 * D:3 * D], np.float32),
        "Wp": np.ascontiguousarray(W_proj, np.float32),
        "bqk": bqk_h,
        "bvp": np.ascontiguousarray(bvp_h, np.float32),
        "bp": np.ascontiguousarray(np.asarray(b_proj).reshape(1, D), np.float32),
        "dlut": dlut_h,
        "dlv": dlv_h,
        "selm": selm_h,
        "zb2init": zb2_h,
    }


def kernel(x, W_attn, b_attn, W_proj, b_proj, lut_k, lut_v):
    x = np.asarray(x, np.float32)
    shared = _host_prep(np.asarray(W_attn, np.float32),
                        np.asarray(b_attn, np.float32),
                        np.asarray(W_proj, np.float32),
                        np.asarray(b_proj, np.float32),
                        np.asarray(lut_k, np.float32),
                        np.asarray(lut_v, np.float32))
    if "nc" not in _CACHE:
        _CACHE["nc"] = build_module()
    nc = _CACHE["nc"]
    in_maps = []
    for b in range(N_CORES):
        m = dict(shared)
        m["xT"] = np.ascontiguousarray(x[b].T)
        in_maps.append(m)
    res = run_bass_kernel_spmd(nc, in_maps, list(range(N_CORES)), trace=TRACE)
    _CACHE["last_result"] = res
    out = np.stack([res.results[b]["OUT"] for b in range(N_CORES)], axis=0)
    return out.astype(np.float32)



# revision 9
# speedup vs baseline: 1.8854x; 1.8854x over previous
import sys

sys.path.insert(0, "/opt/trn_rl_repo")

import math

import numpy as np
import ml_dtypes

import concourse.bass as bass
import concourse.mybir as mybir
import concourse.tile as tile
from concourse import bacc
from concourse.bass_utils import run_bass_kernel_spmd
from concourse.masks import make_identity

F32 = mybir.dt.float32
F32R = mybir.dt.float32r
BF16 = mybir.dt.bfloat16
IDENT = mybir.ActivationFunctionType.Identity
EXPF = mybir.ActivationFunctionType.Exp

B, S, D = 8, 1024, 1024
N_H = 16
REL_K = 16
d_k = D // N_H  # 64
N_CORES = 8
MASKVAL = -1e30

# Score segments per key-block jt: absolute query ranges, each one matmul
# (<=512 cols; f32r needs >=256 cols for full rate).  First segment of each
# block carries the band add; jt7 is padded to 256 cols (768..896 computed
# but never exp'd).
SEGS = {
    0: [(0, 512), (512, 1024)],
    1: [(128, 512), (512, 1024)],
    2: [(256, 512), (512, 1024)],
    3: [(384, 768), (768, 1024)],
    4: [(512, 1024)],
    5: [(640, 1024)],
    6: [(768, 1024)],
    7: [(768, 1024)],
}

ZDP_SZ = 17 * 1040
EWD_BLK = 128 * 144
EWD_SZ = 8 * EWD_BLK
ZB2_BLK = 128 * 161
ZB2_SZ = 8 * ZB2_BLK
ZB1_BLK = 16 * 145
ZB1_SZ = 8 * ZB1_BLK

_CACHE = {}
TRACE = False


def _ap(t, offset, dims):
    return bass.AP(tensor=t, offset=offset, ap=[list(d) for d in dims])


def build_module():
    nc = bacc.Bacc("TRN2", detect_race_conditions=False, num_swdge_queues=4)

    xT = nc.dram_tensor("xT", [D, S], F32R, kind="ExternalInput")
    Wqk = nc.dram_tensor("Wqk", [D, 2 * D], F32R, kind="ExternalInput")
    Wv = nc.dram_tensor("Wv", [D, D], F32R, kind="ExternalInput")
    Wp = nc.dram_tensor("Wp", [D, D], F32R, kind="ExternalInput")
    bqk = nc.dram_tensor("bqk", [128, 16], F32, kind="ExternalInput")
    bvp = nc.dram_tensor("bvp", [1, D], F32R, kind="ExternalInput")
    bp = nc.dram_tensor("bp", [1, D], F32R, kind="ExternalInput")
    dlut = nc.dram_tensor("dlut", [d_k, 16], F32R, kind="ExternalInput")
    dlv = nc.dram_tensor("dlv", [16, d_k], BF16, kind="ExternalInput")
    selm = nc.dram_tensor("selm", [16, 1024], F32R, kind="ExternalInput")
    zb2c = [nc.dram_tensor(f"zb2c{i}", [ZB2_SZ], BF16, kind="ExternalInput")
            for i in range(2)]
    zb1c = [nc.dram_tensor(f"zb1c{i}", [ZB1_SZ], BF16, kind="ExternalInput")
            for i in range(2)]
    OUT = nc.dram_tensor("OUT", [S, D], F32, kind="ExternalOutput")

    zdp = [nc.dram_tensor(f"zdp{i}", [ZDP_SZ], BF16) for i in range(2)]
    ewd = [nc.dram_tensor(f"ewd{i}", [EWD_SZ], BF16) for i in range(2)]

    with tile.TileContext(nc) as tc, nc.allow_low_precision(
            reason="bf16 attention weights/corrections within tolerance"), \
            tc.tile_pool(name="persist", bufs=1) as pers:
        # ---- constants ----
        ident = pers.tile([128, 128], F32)
        make_identity(nc, ident[:])
        identB = pers.tile([128, 128], BF16)
        nc.vector.tensor_copy(identB[:], ident[:])
        dlut_sb = pers.tile([128, 16], F32R)
        nc.sync.dma_start(out=dlut_sb[0:64, :], in_=dlut[:])
        nc.sync.dma_start(out=dlut_sb[64:128, :], in_=dlut[:])
        dlv_sb = pers.tile([16, d_k], BF16)
        nc.sync.dma_start(out=dlv_sb[:], in_=dlv[:])
        selm_sb = pers.tile([16, 1024], F32R)
        nc.sync.dma_start(out=selm_sb[:], in_=selm[:])
        bqk_sb = pers.tile([128, 16], F32)
        nc.sync.dma_start(out=bqk_sb[:], in_=bqk[:])
        bvp_sb = pers.tile([1, D], F32R)
        nc.sync.dma_start(out=bvp_sb[:], in_=bvp[:])
        bp_sb = pers.tile([1, D], F32R)
        nc.sync.dma_start(out=bp_sb[:], in_=bp[:])
        ones1f = pers.tile([1, 128], F32)
        nc.vector.memset(ones1f[:], 1.0)
        ones1 = pers.tile([1, 128], F32R)
        nc.vector.tensor_copy(ones1[:], ones1f[:])
        onescol = pers.tile([128, 16], BF16)
        nc.vector.memset(onescol[:], 1.0)

        denoms = pers.tile([16, S], F32)

        dpT_t = [pers.tile([16, 1040], BF16, name=f"dpT{i}", tag=f"dpT{i}")
                 for i in range(2)]
        for i in range(2):
            nc.vector.memset(dpT_t[i][:, 1024:1040], 0.0)
        dpSh_t = [pers.tile([16, 1040], BF16, name=f"dpSh{i}", tag=f"dpSh{i}")
                  for i in range(2)]
        dpS_t = [pers.tile([128, 128], BF16, name=f"dpS{i}", tag=f"dpS{i}")
                 for i in range(2)]
        esT_t = [pers.tile([16, 1024], BF16, name=f"esT{i}", tag=f"esT{i}")
                 for i in range(2)]

        xT_sb = []
        for d in range(8):
            t = pers.tile([128, S], F32R, name=f"xTt{d}", tag=f"xT{d}")
            nc.gpsimd.dma_start(out=t[:], in_=xT[128 * d:128 * (d + 1), :])
            xT_sb.append(t)

        vhat_sb = [pers.tile([128, 16 * 65], BF16, name=f"vh{jt}",
                             tag=f"vh{jt}") for jt in range(8)]
        pair_sb = [pers.tile([128, S], F32R, name=f"pair{hp}",
                             tag=f"pair{hp}") for hp in range(8)]

        with (
            tc.tile_pool(name="wqkp", bufs=2) as wqkp,
            tc.tile_pool(name="qkp", bufs=2) as qkp,
            tc.tile_pool(name="bandp", bufs=3) as bandp,
            tc.tile_pool(name="ps_s", bufs=3, space="PSUM") as ps_s,
            tc.tile_pool(name="ps_tr", bufs=1, space="PSUM") as ps_tr,
        ):
            wq_tiles = {}
            qk_tiles = {}
            band_tiles = {}
            dt_tiles = {}

            def emit_wload(hp):
                ws = []
                for sec, ft in ((0, hp), (1, 8 + hp)):
                    w = wqkp.tile([128, 1024], F32R, name=f"wqk{hp}_{sec}",
                                  tag=f"wqk{sec}")
                    src = _ap(Wqk[:].tensor, 128 * ft,
                              [[2 * D, 128], [128 * 2 * D, 8], [1, 128]])
                    dst = _ap(w[:].tensor, 0,
                              [[1024, 128], [128, 8], [1, 128]])
                    nc.gpsimd.dma_start(out=dst, in_=src)
                    ws.append(w)
                wq_tiles[hp] = ws

            def emit_qkproj(hp):
                ws = wq_tiles.pop(hp)
                pair = []
                for sec, ft in ((0, hp), (1, 8 + hp)):
                    dstt = qkp.tile([128, S], F32R, name=f"qk{hp}_{sec}",
                                    tag=f"qk{sec}")
                    for tch in range(2):
                        ps = ps_s.tile([128, 512], F32, name="psqk", tag="s")
                        for d in range(8):
                            nc.tensor.matmul(
                                ps[:],
                                ws[sec][:, 128 * d:128 * (d + 1)],
                                xT_sb[d][:, 512 * tch:512 * (tch + 1)],
                                start=(d == 0), stop=(d == 7),
                            )
                        nc.scalar.activation(
                            dstt[:, 512 * tch:512 * (tch + 1)], ps[:], IDENT,
                            bias=bqk_sb[:, ft:ft + 1], scale=1.0)
                    pair.append(dstt)
                qk_tiles[hp] = pair

            def emit_band_stage1(h):
                hp, hh = h // 2, h % 2
                po = 64 * hh
                par = h % 2
                qT = qk_tiles[hp][0]
                for c in range(2):
                    psdp = ps_s.tile([128, 512], F32, name="psdp", tag="s")
                    nc.tensor.matmul(
                        psdp[0:16, :], dlut_sb[po:po + 64, :],
                        qT[po:po + 64, 512 * c:512 * (c + 1)],
                        start=True, stop=True)
                    nc.vector.tensor_copy(
                        dpT_t[par][:, 512 * c:512 * (c + 1)], psdp[0:16, :])
                nc.gpsimd.dma_start(
                    out=_ap(zdp[par][:].tensor, 0, [[1040, 16], [1, 1040]]),
                    in_=dpT_t[par][:])
                nc.sync.dma_start(
                    out=dpSh_t[par][:],
                    in_=_ap(zdp[par][:].tensor, 0, [[1041, 16], [1, 1040]]))

            def emit_band_stage2(h):
                par = h % 2
                psG = ps_tr.tile([128, 128], BF16, name="psG", tag="tr")
                for jt in range(8):
                    j0 = 128 * jt
                    nc.tensor.transpose(
                        psG[:, 16 * jt:16 * (jt + 1)],
                        dpSh_t[par][:, j0:j0 + 128],
                        identB[0:16, 0:16])
                nc.vector.tensor_copy(dpS_t[par][:], psG[:])
                nc.gpsimd.dma_start(
                    out=_ap(zb2c[par][:].tensor, 0,
                            [[162, 128], [ZB2_BLK, 8], [1, 16]]),
                    in_=_ap(dpS_t[par][:].tensor, 0,
                            [[128, 128], [16, 8], [1, 16]]))
                band = bandp.tile([128, 8 * 144], BF16, name=f"band{h}",
                                  tag="band")
                nc.sync.dma_start(
                    out=_ap(band[:].tensor, 0,
                            [[8 * 144, 128], [144, 8], [1, 144]]),
                    in_=_ap(zb2c[par][:].tensor, 0,
                            [[161, 128], [ZB2_BLK, 8], [1, 144]]))
                band_tiles[h] = band

            # ---- bootstrap: qkproj(0) + band stage1 before vproj ----
            emit_wload(0)
            emit_wload(1)
            emit_qkproj(0)
            emit_band_stage1(0)
            emit_band_stage1(1)

            # ---- v projection (PE-dense; hides band bootstrap latency) ----
            with tc.tile_pool(name="wv", bufs=1) as wvp:
                Wv_sb = []
                for d in range(8):
                    t = wvp.tile([128, D], F32R, name=f"wvt{d}", tag=f"wv{d}")
                    nc.gpsimd.dma_start(out=t[:],
                                        in_=Wv[128 * d:128 * (d + 1), :])
                    Wv_sb.append(t)
                for tt in range(8):
                    if tt == 4:
                        emit_band_stage2(0)
                        emit_band_stage2(1)
                    vt = vhat_sb[tt]
                    ones_ap = _ap(vt[:].tensor, 64, [[16 * 65, 128], [65, 16]])
                    nc.vector.tensor_copy(ones_ap, onescol[:])
                    for fc in range(2):
                        ps = ps_s.tile([128, 512], F32, name="psv", tag="s")
                        for d in range(8):
                            nc.tensor.matmul(
                                ps[:],
                                xT_sb[d][:, 128 * tt:128 * (tt + 1)],
                                Wv_sb[d][:, 512 * fc:512 * (fc + 1)],
                                start=(d == 0), stop=False,
                            )
                        nc.tensor.matmul(
                            ps[:], ones1[:], bvp_sb[:, 512 * fc:512 * (fc + 1)],
                            start=False, stop=True,
                        )
                        src = _ap(ps[:].tensor, ps[:].offset,
                                  [[512, 128], [64, 8], [1, 64]])
                        dst = _ap(vt[:].tensor, 65 * 8 * fc,
                                  [[16 * 65, 128], [65, 8], [1, 64]])
                        nc.scalar.copy(dst, src)

            # ---- attention-only pools + head loop ----
            with (
                tc.tile_pool(name="expp", bufs=2) as expp,
                tc.tile_pool(name="dtp", bufs=2) as dtp,
                tc.tile_pool(name="sm", bufs=2) as sm,
                tc.tile_pool(name="outtp", bufs=2) as outtp,
                tc.tile_pool(name="ps_o", bufs=1, space="PSUM") as ps_o,
                tc.tile_pool(name="ps_dt", bufs=1, space="PSUM") as ps_dt,
            ):
                dt_ps_tile = ps_dt.tile([128, 1024], F32, name="dtps",
                                        tag="dt")

                def emit_scores(h):
                    hp, hh = h // 2, h % 2
                    po = 64 * hh
                    par = h % 2
                    qT = qk_tiles[hp][0]
                    kT = qk_tiles[hp][1]
                    band = band_tiles.pop(h)
                    eA = expp.tile([128, 4096], BF16, name=f"eA{h}", tag="eA")
                    eB = expp.tile([128, 4096], BF16, name=f"eB{h}", tag="eB")
                    pso = ps_o.tile([65, 1024], F32, name="pso", tag="pso")

                    def slot(jt):
                        return (eA, 1024 * jt) if jt < 4 else \
                            (eB, 1024 * (jt - 4))

                    pso_pending = []

                    def flush_pso(final=False):
                        for i, (jt2, a, b) in enumerate(pso_pending):
                            j02 = 128 * jt2
                            t2, o2 = slot(jt2)
                            nc.tensor.matmul(
                                pso[:, a:b],
                                vhat_sb[jt2][:, 65 * h:65 * h + 65],
                                t2[:, o2 + a - j02:o2 + b - j02],
                                start=(jt2 == 0),
                                stop=(final and i == len(pso_pending) - 1),
                                skip_group_check=True)
                        pso_pending.clear()

                    for jt in range(8):
                        j0 = 128 * jt
                        t, o = slot(jt)
                        win = min(144, S - j0)
                        for si, (a, b) in enumerate(SEGS[jt]):
                            psS = ps_s.tile([128, 512], F32, name="psS",
                                            tag="s")
                            first = (si == 0)
                            nc.tensor.matmul(
                                psS[:, 0:b - a],
                                kT[po:po + 64, j0:j0 + 128],
                                qT[po:po + 64, a:b],
                                start=True, stop=not first,
                                skip_group_check=True)
                            if first:
                                c0 = j0 - a  # 0 except jt7 -> 128
                                bw = win if jt < 7 else 128
                                nc.tensor.matmul(
                                    psS[:, c0:c0 + bw], identB[:],
                                    band[:, 144 * jt:144 * jt + bw],
                                    start=False, stop=True,
                                    skip_group_check=True)
                                flush_pso()
                                nc.scalar.activation(
                                    t[:, o:o + bw], psS[:, c0:c0 + bw], EXPF)
                                if jt == 7:
                                    # pad cols 128..144 of the jt7 slot: the
                                    # ewd write reads 144 cols per slot
                                    nc.vector.memset(t[:, o + 128:o + 144],
                                                     0.0)
                                if jt < 7 and b - a > win:
                                    nc.scalar.activation(
                                        t[:, o + win:o + (b - a)],
                                        psS[:, win:b - a], EXPF)
                            else:
                                flush_pso()
                                nc.scalar.activation(
                                    t[:, o + a - j0:o + b - j0],
                                    psS[:, 0:b - a], EXPF)
                        lo = j0 if jt < 7 else 896
                        if lo < 512:
                            pso_pending.append((jt, lo, 512))
                            pso_pending.append((jt, 512, 1024))
                        else:
                            pso_pending.append((jt, lo, 1024))
                        if jt == 3:
                            nc.gpsimd.dma_start(
                                out=_ap(ewd[par][:].tensor, 0,
                                        [[144, 128], [EWD_BLK, 4], [1, 144]]),
                                in_=_ap(eA[:].tensor, 0,
                                        [[4096, 128], [1024, 4], [1, 144]]))
                    flush_pso(final=True)
                    nc.gpsimd.dma_start(
                        out=_ap(ewd[par][:].tensor, 4 * EWD_BLK,
                                [[144, 128], [EWD_BLK, 4], [1, 144]]),
                        in_=_ap(eB[:].tensor, 0,
                                [[4096, 128], [1024, 4], [1, 144]]))
                    return pso

                def emit_esk_read(h):
                    par = h % 2
                    esk = sm.tile([128, 128], BF16, name=f"esk{h}", tag="esk")
                    nc.sync.dma_start(
                        out=_ap(esk[:].tensor, 0,
                                [[128, 128], [16, 8], [1, 16]]),
                        in_=_ap(ewd[par][:].tensor, 0,
                                [[145, 128], [EWD_BLK, 8], [1, 16]]))
                    return esk

                def emit_dt_back(h, esk):
                    par = h % 2
                    for half in range(2):
                        psE = ps_tr.tile([16, 512], BF16, name="psE",
                                         tag="tr")
                        for q in range(4):
                            jt = 4 * half + q
                            nc.tensor.transpose(
                                psE[:, 128 * q:128 * (q + 1)],
                                esk[:, 16 * jt:16 * (jt + 1)],
                                identB[:])
                        nc.vector.tensor_copy(
                            esT_t[par][:, 512 * half:512 * (half + 1)],
                            psE[:])
                    nc.gpsimd.dma_start(
                        out=_ap(zb1c[par][:].tensor, 0,
                                [[146, 16], [ZB1_BLK, 8], [1, 128]]),
                        in_=_ap(esT_t[par][:].tensor, 0,
                                [[1024, 16], [128, 8], [1, 128]]))
                    dt_all = dtp.tile([16, 8 * 144], BF16, name=f"dt{h}",
                                      tag="dt")
                    nc.sync.dma_start(
                        out=_ap(dt_all[:].tensor, 0,
                                [[8 * 144, 16], [144, 8], [1, 144]]),
                        in_=_ap(zb1c[par][:].tensor, 0,
                                [[145, 16], [ZB1_BLK, 8], [1, 144]]))
                    dt_tiles[h] = dt_all

                def emit_dt_mms(h):
                    # per 128-query tile: one start=True matmul from the own
                    # block's window plus a 16-col start=False overlap matmul
                    # from the previous block (always a subrange, so PSUM
                    # accumulate regions stay uniform)
                    hh = h % 2
                    ph = 64 * hh
                    dt_all = dt_tiles.pop(h)
                    for ct in range(8):
                        q0 = 128 * ct
                        nc.tensor.matmul(
                            dt_ps_tile[ph:ph + 64, q0:q0 + 128], dlv_sb[:],
                            dt_all[:, 144 * ct:144 * ct + 128],
                            start=True, stop=False, skip_group_check=True)
                        if ct >= 1:
                            nc.tensor.matmul(
                                dt_ps_tile[ph:ph + 64, q0:q0 + 16], dlv_sb[:],
                                dt_all[:, 144 * (ct - 1) + 128:
                                       144 * (ct - 1) + 144],
                                start=False, stop=(ct == 7),
                                skip_group_check=True)
                    if hh == 1:
                        hp = h // 2
                        nc.vector.tensor_add(pair_sb[hp][:], pair_sb[hp][:],
                                             dt_ps_tile[:])

                def emit_evict(h, pso):
                    hp, hh = h // 2, h % 2
                    po = 64 * hh
                    outT = outtp.tile([65, 1024], F32R, name=f"outT{h}",
                                      tag="outT")
                    nc.scalar.copy(outT[:], pso[:])
                    nc.sync.dma_start(out=pair_sb[hp][po:po + 64, :],
                                      in_=outT[0:64, :])
                    nc.sync.dma_start(out=denoms[h:h + 1, :].bitcast(F32R),
                                      in_=outT[64:65, :])

                for h in range(16):
                    hp, hh = h // 2, h % 2
                    if hh == 0:
                        if hp + 2 < 8:
                            emit_wload(hp + 2)
                        if hp + 1 < 8:
                            emit_qkproj(hp + 1)
                            emit_band_stage1(2 * hp + 2)
                            emit_band_stage1(2 * hp + 3)
                    esk_prev = emit_esk_read(h - 1) if h >= 1 else None
                    pso = emit_scores(h)
                    if h + 2 < 16:
                        emit_band_stage2(h + 2)
                    if h >= 1:
                        emit_dt_back(h - 1, esk_prev)
                    if h >= 2:
                        emit_dt_mms(h - 2)
                    emit_evict(h, pso)
                    if hh == 1:
                        qk_tiles.pop(hp, None)

                esk15 = emit_esk_read(15)
                emit_dt_back(15, esk15)
                emit_dt_mms(14)
                emit_dt_mms(15)

            # ---- normalize ----
            recip = pers.tile([16, S], F32R)
            nc.vector.reciprocal(recip[:], denoms[:])
            with tc.tile_pool(name="ps_n", bufs=2, space="PSUM") as ps_n:
                for hp in range(8):
                    psb = ps_n.tile([128, 1024], F32, name="psb", tag="psb")
                    for c in range(2):
                        nc.tensor.matmul(
                            psb[:, 512 * c:512 * (c + 1)],
                            selm_sb[:, 128 * hp:128 * (hp + 1)],
                            recip[:, 512 * c:512 * (c + 1)],
                            start=True, stop=True)
                    nc.vector.tensor_mul(pair_sb[hp][:], pair_sb[hp][:],
                                         psb[:])

            # ---- final projection ----
            with (
                tc.tile_pool(name="wp", bufs=1) as wpp,
                tc.tile_pool(name="ps_p", bufs=2, space="PSUM") as ps_p,
                tc.tile_pool(name="outp", bufs=2) as outp,
            ):
                Wp_sb = []
                for d in range(8):
                    t = wpp.tile([128, D], F32R, name=f"wpt{d}", tag=f"wp{d}")
                    nc.gpsimd.dma_start(out=t[:],
                                        in_=Wp[128 * d:128 * (d + 1), :])
                    Wp_sb.append(t)
                for tt in range(8):
                    ps = ps_p.tile([128, 1024], F32, name="psp", tag="psp")
                    for fc in range(2):
                        for d in range(8):
                            nc.tensor.matmul(
                                ps[:, 512 * fc:512 * (fc + 1)],
                                pair_sb[d][:, 128 * tt:128 * (tt + 1)],
                                Wp_sb[d][:, 512 * fc:512 * (fc + 1)],
                                start=(d == 0), stop=False,
                            )
                        nc.tensor.matmul(
                            ps[:, 512 * fc:512 * (fc + 1)],
                            ones1[:], bp_sb[:, 512 * fc:512 * (fc + 1)],
                            start=False, stop=True,
                        )
                    ot = outp.tile([128, 1024], F32, name="ot", tag="ot")
                    nc.vector.tensor_copy(ot[:], ps[:])
                    nc.sync.dma_start(out=OUT[128 * tt:128 * (tt + 1), :],
                                      in_=ot[:])

    nc.compile()
    return nc


def _host_prep(W_attn, b_attn, W_proj, b_proj, lut_k, lut_v):
    scale = 1.0 / math.sqrt(d_k)
    Wqk = np.concatenate([W_attn[:, :D], W_attn[:, D:2 * D] * scale], axis=1)
    bq = b_attn[:D]
    bk = b_attn[D:2 * D] * scale
    bqk_h = np.stack([np.concatenate([bq, bk])[128 * ft:128 * (ft + 1)]
                      for ft in range(16)], axis=1).astype(np.float32)
    bvp_h = (b_attn[2 * D:3 * D] + np.tile(lut_v[0], N_H)).reshape(1, D)
    dlut_h = np.stack([(lut_k[16 - u] - lut_k[0]) * scale for u in range(16)],
                      axis=1).astype(np.float32)
    dlv_h = np.stack([lut_v[16 - u] - lut_v[0] for u in range(16)],
                     axis=0).astype(ml_dtypes.bfloat16)
    selm_h = np.zeros((16, 1024), np.float32)
    for hp in range(8):
        for p in range(128):
            selm_h[2 * hp + p // 64, 128 * hp + p] = 1.0
    blk = np.zeros((128, 161), np.float32)
    cols = np.arange(161)[None, :]
    rows = np.arange(128)[:, None]
    blk[cols < rows] = MASKVAL
    zb2c_h = np.tile(blk.reshape(-1), 8).astype(ml_dtypes.bfloat16)
    zb1c_h = np.zeros(ZB1_SZ, ml_dtypes.bfloat16)
    return {
        "Wqk": np.ascontiguousarray(Wqk, np.float32),
        "Wv": np.ascontiguousarray(W_attn[:, 2 * D:3 * D], np.float32),
        "Wp": np.ascontiguousarray(W_proj, np.float32),
        "bqk": bqk_h,
        "bvp": np.ascontiguousarray(bvp_h, np.float32),
        "bp": np.ascontiguousarray(np.asarray(b_proj).reshape(1, D),
                                   np.float32),
        "dlut": dlut_h,
        "dlv": dlv_h,
        "selm": selm_h,
        "zb2c0": zb2c_h,
        "zb2c1": zb2c_h.copy(),
        "zb1c0": zb1c_h,
        "zb1c1": zb1c_h.copy(),
    }


def kernel(x, W_attn, b_attn, W_proj, b_proj, lut_k, lut_v):
    x = np.asarray(x, np.float32)
    shared = _host_prep(np.asarray(W_attn, np.float32),
                        np.asarray(b_attn, np.float32),
                        np.asarray(W_proj, np.float32),
                        np.asarray(b_proj, np.float32),
                        np.asarray(lut_k, np.float32),
                        np.asarray(lut_v, np.float32))
    if "nc" not in _CACHE:
        _CACHE["nc"] = build_module()
    nc = _CACHE["nc"]
    in_maps = []
    for b in range(N_CORES):
        m = dict(shared)
        m["xT"] = np.ascontiguousarray(x[b].T)
        in_maps.append(m)
    res = run_bass_kernel_spmd(nc, in_maps, list(range(N_CORES)), trace=TRACE)
    _CACHE["last_result"] = res
    out = np.stack([res.results[b]["OUT"] for b in range(N_CORES)], axis=0)
    return out.astype(np.float32)


# revision 15
# speedup vs baseline: 2.1021x; 1.1149x over previous
import sys

sys.path.insert(0, "/opt/trn_rl_repo")

import math

import numpy as np
import ml_dtypes

import concourse.bass as bass
import concourse.mybir as mybir
import concourse.tile as tile
from concourse import bacc
from concourse.bass_utils import run_bass_kernel_spmd
from concourse.masks import make_identity

F32 = mybir.dt.float32
F32R = mybir.dt.float32r
BF16 = mybir.dt.bfloat16
IDENT = mybir.ActivationFunctionType.Identity
EXPF = mybir.ActivationFunctionType.Exp

B, S, D = 8, 1024, 1024
N_H = 16
REL_K = 16
d_k = D // N_H  # 64
N_CORES = 8
MASKVAL = -1e30

# Score segments per key-block jt: absolute query ranges, each one matmul
# (<=512 cols; f32r needs >=256 cols for full rate).  First segment of each
# block carries the band add; jt7 is padded to 256 cols (768..896 computed
# but never exp'd).
SEGS = {
    0: [(0, 512), (512, 1024)],
    1: [(128, 512), (512, 1024)],
    2: [(256, 512), (512, 1024)],
    3: [(384, 768), (768, 1024)],
    4: [(512, 1024)],
    5: [(640, 1024)],
    6: [(768, 1024)],
    7: [(768, 1024)],
}

ZDP_SZ = 17 * 1040
EWD_BLK = 128 * 144
EWD_SZ = 8 * EWD_BLK
ZB2_BLK = 128 * 161
ZB2_SZ = 8 * ZB2_BLK
ZB1_BLK = 16 * 145
ZB1_SZ = 8 * ZB1_BLK

_CACHE = {}
TRACE = False


def _ap(t, offset, dims):
    return bass.AP(tensor=t, offset=offset, ap=[list(d) for d in dims])


def build_module():
    nc = bacc.Bacc("TRN2", detect_race_conditions=False, num_swdge_queues=4)

    xT = nc.dram_tensor("xT", [D, S], F32R, kind="ExternalInput")
    Wqk = nc.dram_tensor("Wqk", [D, 2 * D], F32R, kind="ExternalInput")
    Wv = nc.dram_tensor("Wv", [D, D], F32R, kind="ExternalInput")
    Wp = nc.dram_tensor("Wp", [D, D], F32R, kind="ExternalInput")
    bqk = nc.dram_tensor("bqk", [128, 16], F32, kind="ExternalInput")
    bvp = nc.dram_tensor("bvp", [1, D], F32R, kind="ExternalInput")
    bp = nc.dram_tensor("bp", [1, D], F32R, kind="ExternalInput")
    dlut = nc.dram_tensor("dlut", [d_k, 16], F32R, kind="ExternalInput")
    dlv = nc.dram_tensor("dlv", [16, d_k], BF16, kind="ExternalInput")
    selm2 = nc.dram_tensor("selm2", [2, 128], F32R, kind="ExternalInput")
    zb2c = [nc.dram_tensor(f"zb2c{i}", [ZB2_SZ], BF16, kind="ExternalInput")
            for i in range(2)]
    zb1c = [nc.dram_tensor(f"zb1c{i}", [ZB1_SZ], BF16, kind="ExternalInput")
            for i in range(2)]
    OUT = nc.dram_tensor("OUT", [S, D], F32, kind="ExternalOutput")

    zdp = [nc.dram_tensor(f"zdp{i}", [ZDP_SZ], BF16) for i in range(2)]
    ewd = [nc.dram_tensor(f"ewd{i}", [EWD_SZ], BF16) for i in range(2)]

    with tile.TileContext(nc) as tc, nc.allow_low_precision(
            reason="bf16 attention weights/corrections within tolerance"), \
            tc.tile_pool(name="persist", bufs=1) as pers:
        # ---- constants ----
        ident = pers.tile([128, 128], F32)
        make_identity(nc, ident[:])
        identB = pers.tile([128, 128], BF16)
        nc.vector.tensor_copy(identB[:], ident[:])
        dlut_sb = pers.tile([128, 16], F32R)
        nc.sync.dma_start(out=dlut_sb[0:64, :], in_=dlut[:])
        nc.sync.dma_start(out=dlut_sb[64:128, :], in_=dlut[:])
        dlv_sb = pers.tile([16, d_k], BF16)
        nc.sync.dma_start(out=dlv_sb[:], in_=dlv[:])
        selm2_sb = pers.tile([2, 128], F32R)
        nc.sync.dma_start(out=selm2_sb[:], in_=selm2[:])
        bqk_sb = pers.tile([128, 16], F32)
        nc.sync.dma_start(out=bqk_sb[:], in_=bqk[:])
        bvp_sb = pers.tile([1, D], F32R)
        nc.sync.dma_start(out=bvp_sb[:], in_=bvp[:])
        bp_sb = pers.tile([1, D], F32R)
        nc.sync.dma_start(out=bp_sb[:], in_=bp[:])
        ones1f = pers.tile([1, 128], F32)
        nc.vector.memset(ones1f[:], 1.0)
        ones1 = pers.tile([1, 128], F32R)
        nc.vector.tensor_copy(ones1[:], ones1f[:])
        onescol = pers.tile([128, 16], BF16)
        nc.vector.memset(onescol[:], 1.0)

        dpT_t = [pers.tile([16, 1040], BF16, name=f"dpT{i}", tag=f"dpT{i}")
                 for i in range(2)]
        for i in range(2):
            nc.vector.memset(dpT_t[i][:, 1024:1040], 0.0)
        dpSh_t = [pers.tile([16, 1040], BF16, name=f"dpSh{i}", tag=f"dpSh{i}")
                  for i in range(2)]
        dpS_t = [pers.tile([128, 128], BF16, name=f"dpS{i}", tag=f"dpS{i}")
                 for i in range(2)]
        esT_t = [pers.tile([16, 1024], BF16, name=f"esT{i}", tag=f"esT{i}")
                 for i in range(2)]

        xT_sb = []
        for d in range(8):
            t = pers.tile([128, S], F32R, name=f"xTt{d}", tag=f"xT{d}")
            nc.sync.dma_start(out=t[:], in_=xT[128 * d:128 * (d + 1), :])
            xT_sb.append(t)

        vhat_sb = [pers.tile([128, 16 * 65], BF16, name=f"vh{jt}",
                             tag=f"vh{jt}") for jt in range(8)]
        pair_sb = [pers.tile([128, S], F32R, name=f"pair{hp}",
                             tag=f"pair{hp}") for hp in range(8)]

        with (
            tc.tile_pool(name="wqkp", bufs=2) as wqkp,
            tc.tile_pool(name="qkp", bufs=2) as qkp,
            tc.tile_pool(name="bandp", bufs=3) as bandp,
            tc.tile_pool(name="ps_s", bufs=3, space="PSUM") as ps_s,
            tc.tile_pool(name="ps_tr", bufs=1, space="PSUM") as ps_tr,
        ):
            wq_tiles = {}
            qk_tiles = {}
            band_tiles = {}
            dt_tiles = {}
            den_tiles = {}

            def emit_wload(hp):
                ws = []
                for sec, ft in ((0, hp), (1, 8 + hp)):
                    w = wqkp.tile([128, 1024], F32R, name=f"wqk{hp}_{sec}",
                                  tag=f"wqk{sec}")
                    src = _ap(Wqk[:].tensor, 128 * ft,
                              [[2 * D, 128], [128 * 2 * D, 8], [1, 128]])
                    dst = _ap(w[:].tensor, 0,
                              [[1024, 128], [128, 8], [1, 128]])
                    nc.gpsimd.dma_start(out=dst, in_=src)
                    ws.append(w)
                wq_tiles[hp] = ws

            def emit_qkproj(hp):
                ws = wq_tiles.pop(hp)
                pair = []
                for sec, ft in ((0, hp), (1, 8 + hp)):
                    dstt = qkp.tile([128, S], F32R, name=f"qk{hp}_{sec}",
                                    tag=f"qk{sec}")
                    for tch in range(2):
                        ps = ps_s.tile([128, 512], F32, name="psqk", tag="s")
                        for d in range(8):
                            nc.tensor.matmul(
                                ps[:],
                                ws[sec][:, 128 * d:128 * (d + 1)],
                                xT_sb[d][:, 512 * tch:512 * (tch + 1)],
                                start=(d == 0), stop=(d == 7),
                            )
                        nc.scalar.activation(
                            dstt[:, 512 * tch:512 * (tch + 1)], ps[:], IDENT,
                            bias=bqk_sb[:, ft:ft + 1], scale=1.0)
                    pair.append(dstt)
                qk_tiles[hp] = pair

            def emit_band_stage1(h):
                hp, hh = h // 2, h % 2
                po = 64 * hh
                par = h % 2
                qT = qk_tiles[hp][0]
                for c in range(2):
                    psdp = ps_s.tile([128, 512], F32, name="psdp", tag="s")
                    nc.tensor.matmul(
                        psdp[0:16, :], dlut_sb[po:po + 64, :],
                        qT[po:po + 64, 512 * c:512 * (c + 1)],
                        start=True, stop=True)
                    nc.vector.tensor_copy(
                        dpT_t[par][:, 512 * c:512 * (c + 1)], psdp[0:16, :])
                nc.gpsimd.dma_start(
                    out=_ap(zdp[par][:].tensor, 0, [[1040, 16], [1, 1040]]),
                    in_=dpT_t[par][:])
                nc.sync.dma_start(
                    out=dpSh_t[par][:],
                    in_=_ap(zdp[par][:].tensor, 0, [[1041, 16], [1, 1040]]))

            def emit_band_stage2(h):
                par = h % 2
                psG = ps_tr.tile([128, 128], BF16, name="psG", tag="tr")
                for jt in range(8):
                    j0 = 128 * jt
                    nc.tensor.transpose(
                        psG[:, 16 * jt:16 * (jt + 1)],
                        dpSh_t[par][:, j0:j0 + 128],
                        identB[0:16, 0:16])
                nc.vector.tensor_copy(dpS_t[par][:], psG[:])
                nc.gpsimd.dma_start(
                    out=_ap(zb2c[par][:].tensor, 0,
                            [[162, 128], [ZB2_BLK, 8], [1, 16]]),
                    in_=_ap(dpS_t[par][:].tensor, 0,
                            [[128, 128], [16, 8], [1, 16]]))
                band = bandp.tile([128, 8 * 144], BF16, name=f"band{h}",
                                  tag="band")
                nc.sync.dma_start(
                    out=_ap(band[:].tensor, 0,
                            [[8 * 144, 128], [144, 8], [1, 144]]),
                    in_=_ap(zb2c[par][:].tensor, 0,
                            [[161, 128], [ZB2_BLK, 8], [1, 144]]))
                band_tiles[h] = band

            # ---- bootstrap: qkproj(0) + band stage1 before vproj ----
            emit_wload(0)
            emit_wload(1)
            emit_qkproj(0)
            emit_band_stage1(0)
            emit_band_stage1(1)

            # ---- v projection (PE-dense; hides band bootstrap latency) ----
            with tc.tile_pool(name="wv", bufs=1) as wvp:
                Wv_sb = []
                for d in range(8):
                    t = wvp.tile([128, D], F32R, name=f"wvt{d}", tag=f"wv{d}")
                    nc.gpsimd.dma_start(out=t[:],
                                        in_=Wv[128 * d:128 * (d + 1), :])
                    Wv_sb.append(t)
                for tt in range(8):
                    if tt == 4:
                        emit_band_stage2(0)
                        emit_band_stage2(1)
                    vt = vhat_sb[tt]
                    ones_ap = _ap(vt[:].tensor, 64, [[16 * 65, 128], [65, 16]])
                    nc.vector.tensor_copy(ones_ap, onescol[:])
                    for fc in range(2):
                        ps = ps_s.tile([128, 512], F32, name="psv", tag="s")
                        for d in range(8):
                            nc.tensor.matmul(
                                ps[:],
                                xT_sb[d][:, 128 * tt:128 * (tt + 1)],
                                Wv_sb[d][:, 512 * fc:512 * (fc + 1)],
                                start=(d == 0), stop=False,
                            )
                        nc.tensor.matmul(
                            ps[:], ones1[:], bvp_sb[:, 512 * fc:512 * (fc + 1)],
                            start=False, stop=True,
                        )
                        src = _ap(ps[:].tensor, ps[:].offset,
                                  [[512, 128], [64, 8], [1, 64]])
                        dst = _ap(vt[:].tensor, 65 * 8 * fc,
                                  [[16 * 65, 128], [65, 8], [1, 64]])
                        nc.scalar.copy(dst, src)

            # ---- attention-only pools + head loop ----
            with (
                tc.tile_pool(name="expp", bufs=2) as expp,
                tc.tile_pool(name="dtp", bufs=2) as dtp,
                tc.tile_pool(name="sm", bufs=2) as sm,
                tc.tile_pool(name="outtp", bufs=2) as outtp,
                tc.tile_pool(name="denp", bufs=2) as denp,
                tc.tile_pool(name="ps_o", bufs=1, space="PSUM") as ps_o,
                tc.tile_pool(name="ps_dt", bufs=1, space="PSUM") as ps_dt,
            ):
                dt_ps_tile = ps_dt.tile([128, 1024], F32, name="dtps",
                                        tag="dt")

                def emit_scores(h):
                    hp, hh = h // 2, h % 2
                    po = 64 * hh
                    par = h % 2
                    qT = qk_tiles[hp][0]
                    kT = qk_tiles[hp][1]
                    band = band_tiles.pop(h)
                    eA = expp.tile([128, 4096], BF16, name=f"eA{h}", tag="eA")
                    eB = expp.tile([128, 4096], BF16, name=f"eB{h}", tag="eB")
                    pso = ps_o.tile([65, 1024], F32, name="pso", tag="pso")

                    def slot(jt):
                        return (eA, 1024 * jt) if jt < 4 else \
                            (eB, 1024 * (jt - 4))

                    pso_pending = []

                    def flush_pso(upto, final=False):
                        keep = [p for p in pso_pending if p[0] > upto]
                        todo = [p for p in pso_pending if p[0] <= upto]
                        for i, (jt2, a, b) in enumerate(todo):
                            j02 = 128 * jt2
                            t2, o2 = slot(jt2)
                            nc.tensor.matmul(
                                pso[:, a:b],
                                vhat_sb[jt2][:, 65 * h:65 * h + 65],
                                t2[:, o2 + a - j02:o2 + b - j02],
                                start=(jt2 == 0),
                                stop=(final and i == len(todo) - 1),
                                skip_group_check=True)
                        pso_pending.clear()
                        pso_pending.extend(keep)

                    for jt in range(8):
                        j0 = 128 * jt
                        t, o = slot(jt)
                        win = min(144, S - j0)
                        for si, (a, b) in enumerate(SEGS[jt]):
                            psS = ps_s.tile([128, 512], F32, name="psS",
                                            tag="s")
                            first = (si == 0)
                            nc.tensor.matmul(
                                psS[:, 0:b - a],
                                kT[po:po + 64, j0:j0 + 128],
                                qT[po:po + 64, a:b],
                                start=True, stop=not first,
                                skip_group_check=True)
                            if first:
                                c0 = j0 - a  # 0 except jt7 -> 128
                                bw = (b - a - c0) if jt < 7 else 128
                                nc.tensor.matmul(
                                    psS[:, c0:c0 + min(144, bw)], identB[:],
                                    band[:, 144 * jt:144 * jt + min(144, bw)],
                                    start=False, stop=True,
                                    skip_group_check=True)
                                nc.scalar.activation(
                                    t[:, o:o + bw], psS[:, c0:c0 + bw], EXPF)
                                if jt == 7:
                                    # pad cols 128..144 of the jt7 slot: the
                                    # ewd write reads 144 cols per slot
                                    nc.vector.memset(t[:, o + 128:o + 144],
                                                     0.0)
                            else:
                                nc.scalar.activation(
                                    t[:, o + a - j0:o + b - j0],
                                    psS[:, 0:b - a], EXPF)
                        lo = j0 if jt < 7 else 896
                        if lo < 512:
                            pso_pending.append((jt, lo, 512))
                            pso_pending.append((jt, 512, 1024))
                        else:
                            pso_pending.append((jt, lo, 1024))
                        flush_pso(jt - 2)
                        if jt == 3:
                            nc.gpsimd.dma_start(
                                out=_ap(ewd[par][:].tensor, 0,
                                        [[144, 128], [EWD_BLK, 4], [1, 144]]),
                                in_=_ap(eA[:].tensor, 0,
                                        [[4096, 128], [1024, 4], [1, 144]]))
                    flush_pso(7, final=True)
                    nc.gpsimd.dma_start(
                        out=_ap(ewd[par][:].tensor, 4 * EWD_BLK,
                                [[144, 128], [EWD_BLK, 4], [1, 144]]),
                        in_=_ap(eB[:].tensor, 0,
                                [[4096, 128], [1024, 4], [1, 144]]))
                    return pso

                def emit_esk_read(h):
                    par = h % 2
                    esk = sm.tile([128, 128], BF16, name=f"esk{h}", tag="esk")
                    nc.sync.dma_start(
                        out=_ap(esk[:].tensor, 0,
                                [[128, 128], [16, 8], [1, 16]]),
                        in_=_ap(ewd[par][:].tensor, 0,
                                [[145, 128], [EWD_BLK, 8], [1, 16]]))
                    return esk

                def emit_dt_back(h, esk):
                    par = h % 2
                    for half in range(2):
                        psE = ps_tr.tile([16, 512], BF16, name="psE",
                                         tag="tr")
                        for q in range(4):
                            jt = 4 * half + q
                            nc.tensor.transpose(
                                psE[:, 128 * q:128 * (q + 1)],
                                esk[:, 16 * jt:16 * (jt + 1)],
                                identB[:])
                        nc.vector.tensor_copy(
                            esT_t[par][:, 512 * half:512 * (half + 1)],
                            psE[:])
                    nc.gpsimd.dma_start(
                        out=_ap(zb1c[par][:].tensor, 0,
                                [[146, 16], [ZB1_BLK, 8], [1, 128]]),
                        in_=_ap(esT_t[par][:].tensor, 0,
                                [[1024, 16], [128, 8], [1, 128]]))
                    dt_all = dtp.tile([16, 8 * 144], BF16, name=f"dt{h}",
                                      tag="dt")
                    nc.sync.dma_start(
                        out=_ap(dt_all[:].tensor, 0,
                                [[8 * 144, 16], [144, 8], [1, 144]]),
                        in_=_ap(zb1c[par][:].tensor, 0,
                                [[145, 16], [ZB1_BLK, 8], [1, 144]]))
                    dt_tiles[h] = dt_all

                def emit_dt_mms(h):
                    # per 128-query tile: one start=True matmul from the own
                    # block's window plus a 16-col start=False overlap matmul
                    # from the previous block (always a subrange, so PSUM
                    # accumulate regions stay uniform)
                    hh = h % 2
                    ph = 64 * hh
                    dt_all = dt_tiles.pop(h)
                    for ct in range(8):
                        q0 = 128 * ct
                        nc.tensor.matmul(
                            dt_ps_tile[ph:ph + 64, q0:q0 + 128], dlv_sb[:],
                            dt_all[:, 144 * ct:144 * ct + 128],
                            start=True, stop=False, skip_group_check=True)
                        if ct >= 1:
                            nc.tensor.matmul(
                                dt_ps_tile[ph:ph + 64, q0:q0 + 16], dlv_sb[:],
                                dt_all[:, 144 * (ct - 1) + 128:
                                       144 * (ct - 1) + 144],
                                start=False, stop=(ct == 7),
                                skip_group_check=True)
                    if hh == 1:
                        hp = h // 2
                        nc.vector.tensor_add(pair_sb[hp][:], pair_sb[hp][:],
                                             dt_ps_tile[:])
                        # inline normalize for this pair
                        den2 = den_tiles.pop(hp)
                        recip2 = sm.tile([2, 1024], F32R, name=f"rc{hp}",
                                         tag="rc")
                        nc.vector.reciprocal(recip2[:], den2[:])
                        for c in range(2):
                            psb = ps_s.tile([128, 512], F32, name="psb",
                                            tag="s")
                            nc.tensor.matmul(
                                psb[:], selm2_sb[:],
                                recip2[:, 512 * c:512 * (c + 1)],
                                start=True, stop=True)
                            nc.vector.tensor_mul(
                                pair_sb[hp][:, 512 * c:512 * (c + 1)],
                                pair_sb[hp][:, 512 * c:512 * (c + 1)],
                                psb[:])

                def emit_evict(h, pso):
                    hp, hh = h // 2, h % 2
                    po = 64 * hh
                    outT = outtp.tile([65, 1024], F32R, name=f"outT{h}",
                                      tag="outT")
                    nc.vector.tensor_copy(outT[:], pso[:])
                    nc.sync.dma_start(out=pair_sb[hp][po:po + 64, :],
                                      in_=outT[0:64, :])
                    if hh == 0:
                        den_tiles[hp] = denp.tile([2, 1024], F32R,
                                                  name=f"den{hp}", tag="den")
                    nc.sync.dma_start(out=den_tiles[hp][hh:hh + 1, :],
                                      in_=outT[64:65, :])

                for h in range(16):
                    hp, hh = h // 2, h % 2
                    if hh == 0:
                        if hp + 2 < 8:
                            emit_wload(hp + 2)
                        if hp + 1 < 8:
                            emit_qkproj(hp + 1)
                            emit_band_stage1(2 * hp + 2)
                            emit_band_stage1(2 * hp + 3)
                    esk_prev = emit_esk_read(h - 1) if h >= 1 else None
                    pso = emit_scores(h)
                    if h + 2 < 16:
                        emit_band_stage2(h + 2)
                    if h >= 1:
                        emit_dt_back(h - 1, esk_prev)
                    if h >= 2:
                        emit_dt_mms(h - 2)
                    emit_evict(h, pso)
                    if hh == 1:
                        qk_tiles.pop(hp, None)

                esk15 = emit_esk_read(15)
                emit_dt_back(15, esk15)
                emit_dt_mms(14)
                emit_dt_mms(15)

            # ---- final projection ----
            with (
                tc.tile_pool(name="wp", bufs=1) as wpp,
                tc.tile_pool(name="ps_p", bufs=2, space="PSUM") as ps_p,
                tc.tile_pool(name="outp", bufs=2) as outp,
            ):
                Wp_sb = []
                for d in range(8):
                    t = wpp.tile([128, D], F32R, name=f"wpt{d}", tag=f"wp{d}")
                    nc.gpsimd.dma_start(out=t[:],
                                        in_=Wp[128 * d:128 * (d + 1), :])
                    Wp_sb.append(t)
                for tt in range(8):
                    ps = ps_p.tile([128, 1024], F32, name="psp", tag="psp")
                    for fc in range(2):
                        for d in range(8):
                            nc.tensor.matmul(
                                ps[:, 512 * fc:512 * (fc + 1)],
                                pair_sb[d][:, 128 * tt:128 * (tt + 1)],
                                Wp_sb[d][:, 512 * fc:512 * (fc + 1)],
                                start=(d == 0), stop=False,
                            )
                        nc.tensor.matmul(
                            ps[:, 512 * fc:512 * (fc + 1)],
                            ones1[:], bp_sb[:, 512 * fc:512 * (fc + 1)],
                            start=False, stop=True,
                        )
                    ot = outp.tile([128, 1024], F32, name="ot", tag="ot")
                    nc.vector.tensor_copy(ot[:], ps[:])
                    nc.sync.dma_start(out=OUT[128 * tt:128 * (tt + 1), :],
                                      in_=ot[:])

    nc.compile()
    return nc


def _host_prep(W_attn, b_attn, W_proj, b_proj, lut_k, lut_v):
    scale = 1.0 / math.sqrt(d_k)
    Wqk = np.concatenate([W_attn[:, :D], W_attn[:, D:2 * D] * scale], axis=1)
    bq = b_attn[:D]
    bk = b_attn[D:2 * D] * scale
    bqk_h = np.stack([np.concatenate([bq, bk])[128 * ft:128 * (ft + 1)]
                      for ft in range(16)], axis=1).astype(np.float32)
    bvp_h = (b_attn[2 * D:3 * D] + np.tile(lut_v[0], N_H)).reshape(1, D)
    dlut_h = np.stack([(lut_k[16 - u] - lut_k[0]) * scale for u in range(16)],
                      axis=1).astype(np.float32)
    dlv_h = np.stack([lut_v[16 - u] - lut_v[0] for u in range(16)],
                     axis=0).astype(ml_dtypes.bfloat16)
    selm2_h = np.zeros((2, 128), np.float32)
    for p in range(128):
        selm2_h[p // 64, p] = 1.0
    blk = np.zeros((128, 161), np.float32)
    cols = np.arange(161)[None, :]
    rows = np.arange(128)[:, None]
    blk[cols < rows] = MASKVAL
    zb2c_h = np.tile(blk.reshape(-1), 8).astype(ml_dtypes.bfloat16)
    zb1c_h = np.zeros(ZB1_SZ, ml_dtypes.bfloat16)
    return {
        "Wqk": np.ascontiguousarray(Wqk, np.float32),
        "Wv": np.ascontiguousarray(W_attn[:, 2 * D:3 * D], np.float32),
        "Wp": np.ascontiguousarray(W_proj, np.float32),
        "bqk": bqk_h,
        "bvp": np.ascontiguousarray(bvp_h, np.float32),
        "bp": np.ascontiguousarray(np.asarray(b_proj).reshape(1, D),
                                   np.float32),
        "dlut": dlut_h,
        "dlv": dlv_h,
        "selm2": selm2_h,
        "zb2c0": zb2c_h,
        "zb2c1": zb2c_h.copy(),
        "zb1c0": zb1c_h,
        "zb1c1": zb1c_h.copy(),
    }


def kernel(x, W_attn, b_attn, W_proj, b_proj, lut_k, lut_v):
    x = np.asarray(x, np.float32)
    shared = _host_prep(np.asarray(W_attn, np.float32),
                        np.asarray(b_attn, np.float32),
                        np.asarray(W_proj, np.float32),
                        np.asarray(b_proj, np.float32),
                        np.asarray(lut_k, np.float32),
                        np.asarray(lut_v, np.float32))
    if "nc" not in _CACHE:
        _CACHE["nc"] = build_module()
    nc = _CACHE["nc"]
    in_maps = []
    for b in range(N_CORES):
        m = dict(shared)
        m["xT"] = np.ascontiguousarray(x[b].T)
        in_maps.append(m)
    res = run_bass_kernel_spmd(nc, in_maps, list(range(N_CORES)), trace=TRACE)
    _CACHE["last_result"] = res
    out = np.stack([res.results[b]["OUT"] for b in range(N_CORES)], axis=0)
    return out.astype(np.float32)


# revision 23
# speedup vs baseline: 2.1810x; 1.0375x over previous
import sys

sys.path.insert(0, "/opt/trn_rl_repo")

import math

import numpy as np
import ml_dtypes

import concourse.bass as bass
import concourse.mybir as mybir
import concourse.tile as tile
from concourse import bacc
from concourse.bass_utils import run_bass_kernel_spmd
from concourse.masks import make_identity

F32 = mybir.dt.float32
F32R = mybir.dt.float32r
BF16 = mybir.dt.bfloat16
IDENT = mybir.ActivationFunctionType.Identity
EXPF = mybir.ActivationFunctionType.Exp

B, S, D = 8, 1024, 1024
N_H = 16
REL_K = 16
d_k = D // N_H  # 64
N_CORES = 8
MASKVAL = -1e30

# Score segments per key-block jt: absolute query ranges, each one matmul
# (<=512 cols; f32r needs >=256 cols for full rate).  First segment of each
# block carries the band add; jt7 is padded to 256 cols (768..896 computed
# but never exp'd).
SEGS = {
    0: [(0, 512), (512, 1024)],
    1: [(128, 512), (512, 1024)],
    2: [(256, 512), (512, 1024)],
    3: [(384, 768), (768, 1024)],
    4: [(512, 1024)],
    5: [(640, 1024)],
    6: [(768, 1024)],
    7: [(896, 1024)],
}

ZDP_SZ = 17 * 1040
EWD_BLK = 128 * 144
EWD_SZ = 8 * EWD_BLK
ZB2_BLK = 128 * 161
ZB2_SZ = 8 * ZB2_BLK
ZB1_BLK = 16 * 145
ZB1_SZ = 8 * ZB1_BLK

_CACHE = {}
TRACE = False


def _ap(t, offset, dims):
    return bass.AP(tensor=t, offset=offset, ap=[list(d) for d in dims])


def build_module():
    nc = bacc.Bacc("TRN2", detect_race_conditions=False, num_swdge_queues=4)

    xT = nc.dram_tensor("xT", [D, S], BF16, kind="ExternalInput")
    Wqk = nc.dram_tensor("Wqk", [D, 2 * D], BF16, kind="ExternalInput")
    Wv = nc.dram_tensor("Wv", [D, D], BF16, kind="ExternalInput")
    Wp = nc.dram_tensor("Wp", [D, D], BF16, kind="ExternalInput")
    bqk = nc.dram_tensor("bqk", [128, 16], F32, kind="ExternalInput")
    bvp = nc.dram_tensor("bvp", [1, D], BF16, kind="ExternalInput")
    bp = nc.dram_tensor("bp", [1, D], BF16, kind="ExternalInput")
    dlut = nc.dram_tensor("dlut", [d_k, 16], BF16, kind="ExternalInput")
    dlv = nc.dram_tensor("dlv", [16, d_k], BF16, kind="ExternalInput")
    selm2 = nc.dram_tensor("selm2", [2, 128], F32R, kind="ExternalInput")
    zb2c = [nc.dram_tensor(f"zb2c{i}", [ZB2_SZ], BF16, kind="ExternalInput")
            for i in range(2)]
    zb1c = [nc.dram_tensor(f"zb1c{i}", [ZB1_SZ], BF16, kind="ExternalInput")
            for i in range(2)]
    OUT = nc.dram_tensor("OUT", [S, D], F32, kind="ExternalOutput")

    zdp = [nc.dram_tensor(f"zdp{i}", [ZDP_SZ], BF16) for i in range(2)]
    ewd = [nc.dram_tensor(f"ewd{i}", [EWD_SZ], BF16) for i in range(2)]

    with tile.TileContext(nc) as tc, nc.allow_low_precision(
            reason="bf16 attention weights/corrections within tolerance"), \
            tc.tile_pool(name="persist", bufs=1) as pers:
        # ---- xT first: the first qkproj matmuls gate the whole pipeline ----
        xT_sb = []
        for d in range(8):
            t = pers.tile([128, S], BF16, name=f"xTt{d}", tag=f"xT{d}")
            nc.sync.dma_start(out=t[:], in_=xT[128 * d:128 * (d + 1), :])
            xT_sb.append(t)

        # ---- constants (scalar queue; keep SP free for xT) ----
        ident = pers.tile([128, 128], F32)
        make_identity(nc, ident[:])
        identB = pers.tile([128, 128], BF16)
        nc.vector.tensor_copy(identB[:], ident[:])
        dlut_sb = pers.tile([128, 16], BF16)
        nc.scalar.dma_start(out=dlut_sb[0:64, :], in_=dlut[:])
        nc.scalar.dma_start(out=dlut_sb[64:128, :], in_=dlut[:])
        dlv_sb = pers.tile([16, d_k], BF16)
        nc.scalar.dma_start(out=dlv_sb[:], in_=dlv[:])
        selm2_sb = pers.tile([2, 128], F32R)
        nc.scalar.dma_start(out=selm2_sb[:], in_=selm2[:])
        bqk_sb = pers.tile([128, 16], F32)
        nc.scalar.dma_start(out=bqk_sb[:], in_=bqk[:])
        bvp_sb = pers.tile([1, D], BF16)
        nc.scalar.dma_start(out=bvp_sb[:], in_=bvp[:])
        bp_sb = pers.tile([1, D], BF16)
        nc.scalar.dma_start(out=bp_sb[:], in_=bp[:])
        ones1 = pers.tile([1, 128], BF16)
        nc.vector.memset(ones1[:], 1.0)
        onescol = pers.tile([128, 16], BF16)
        nc.vector.memset(onescol[:], 1.0)

        dpT_t = [pers.tile([16, 1040], BF16, name=f"dpT{i}", tag=f"dpT{i}")
                 for i in range(2)]
        for i in range(2):
            nc.vector.memset(dpT_t[i][:, 1024:1040], 0.0)
        dpSh_t = [pers.tile([16, 1040], BF16, name=f"dpSh{i}", tag=f"dpSh{i}")
                  for i in range(2)]
        dpS_t = [pers.tile([128, 128], BF16, name=f"dpS{i}", tag=f"dpS{i}")
                 for i in range(2)]
        esT_t = [pers.tile([16, 1024], BF16, name=f"esT{i}", tag=f"esT{i}")
                 for i in range(2)]

        vhat_sb = [pers.tile([128, 16 * 65], BF16, name=f"vh{jt}",
                             tag=f"vh{jt}") for jt in range(8)]
        pair_sb = [pers.tile([128, S], BF16, name=f"pair{hp}",
                             tag=f"pair{hp}") for hp in range(8)]

        with (
            tc.tile_pool(name="wqkp", bufs=2) as wqkp,
            tc.tile_pool(name="qkp", bufs=2) as qkp,
            tc.tile_pool(name="bandp", bufs=3) as bandp,
            tc.tile_pool(name="ps_s", bufs=3, space="PSUM") as ps_s,
            tc.tile_pool(name="ps_tr", bufs=1, space="PSUM") as ps_tr,
        ):
            wq_tiles = {}
            qk_tiles = {}
            band_tiles = {}
            dt_tiles = {}
            den_tiles = {}

            def emit_wload(hp):
                ws = []
                for sec, ft in ((0, hp), (1, 8 + hp)):
                    w = wqkp.tile([128, 1024], BF16, name=f"wqk{hp}_{sec}",
                                  tag=f"wqk{sec}")
                    src = _ap(Wqk[:].tensor, 128 * ft,
                              [[2 * D, 128], [128 * 2 * D, 8], [1, 128]])
                    dst = _ap(w[:].tensor, 0,
                              [[1024, 128], [128, 8], [1, 128]])
                    nc.gpsimd.dma_start(out=dst, in_=src)
                    ws.append(w)
                wq_tiles[hp] = ws

            def emit_qkproj(hp):
                ws = wq_tiles.pop(hp)
                pair = []
                for sec, ft in ((0, hp), (1, 8 + hp)):
                    dstt = qkp.tile([128, S], BF16, name=f"qk{hp}_{sec}",
                                    tag=f"qk{sec}")
                    for tch in range(2):
                        ps = ps_s.tile([128, 512], F32, name="psqk", tag="s")
                        for d in range(8):
                            nc.tensor.matmul(
                                ps[:],
                                ws[sec][:, 128 * d:128 * (d + 1)],
                                xT_sb[d][:, 512 * tch:512 * (tch + 1)],
                                start=(d == 0), stop=(d == 7),
                            )
                        nc.scalar.activation(
                            dstt[:, 512 * tch:512 * (tch + 1)], ps[:], IDENT,
                            bias=bqk_sb[:, ft:ft + 1], scale=1.0)
                    pair.append(dstt)
                qk_tiles[hp] = pair

            def emit_band_stage1(h):
                hp, hh = h // 2, h % 2
                po = 64 * hh
                par = h % 2
                qT = qk_tiles[hp][0]
                for c in range(2):
                    psdp = ps_s.tile([128, 512], F32, name="psdp", tag="s")
                    nc.tensor.matmul(
                        psdp[0:16, :], dlut_sb[po:po + 64, :],
                        qT[po:po + 64, 512 * c:512 * (c + 1)],
                        start=True, stop=True)
                    nc.vector.tensor_copy(
                        dpT_t[par][:, 512 * c:512 * (c + 1)], psdp[0:16, :])
                nc.gpsimd.dma_start(
                    out=_ap(zdp[par][:].tensor, 0, [[1040, 16], [1, 1040]]),
                    in_=dpT_t[par][:])
                nc.sync.dma_start(
                    out=dpSh_t[par][:],
                    in_=_ap(zdp[par][:].tensor, 0, [[1041, 16], [1, 1040]]))

            def emit_band_stage2(h):
                par = h % 2
                psG = ps_tr.tile([128, 128], BF16, name="psG", tag="tr")
                for jt in range(8):
                    j0 = 128 * jt
                    nc.tensor.transpose(
                        psG[:, 16 * jt:16 * (jt + 1)],
                        dpSh_t[par][:, j0:j0 + 128],
                        identB[0:16, 0:16])
                nc.vector.tensor_copy(dpS_t[par][:], psG[:])
                nc.gpsimd.dma_start(
                    out=_ap(zb2c[par][:].tensor, 0,
                            [[162, 128], [ZB2_BLK, 8], [1, 16]]),
                    in_=_ap(dpS_t[par][:].tensor, 0,
                            [[128, 128], [16, 8], [1, 16]]))
                band = bandp.tile([128, 8 * 144], BF16, name=f"band{h}",
                                  tag="band")
                nc.sync.dma_start(
                    out=_ap(band[:].tensor, 0,
                            [[8 * 144, 128], [144, 8], [1, 144]]),
                    in_=_ap(zb2c[par][:].tensor, 0,
                            [[161, 128], [ZB2_BLK, 8], [1, 144]]))
                band_tiles[h] = band

            # ---- bootstrap: qkproj(0) + band stage1 before vproj ----
            emit_wload(0)
            emit_wload(1)
            emit_qkproj(0)
            emit_band_stage1(0)
            emit_band_stage1(1)

            # ---- v projection (PE-dense; hides band bootstrap latency) ----
            with tc.tile_pool(name="wv", bufs=1) as wvp:
                Wv_sb = []
                for d in range(8):
                    t = wvp.tile([128, D], BF16, name=f"wvt{d}", tag=f"wv{d}")
                    nc.gpsimd.dma_start(out=t[:],
                                        in_=Wv[128 * d:128 * (d + 1), :])
                    Wv_sb.append(t)
                for tt in range(8):
                    if tt == 4:
                        emit_band_stage2(0)
                        emit_band_stage2(1)
                    vt = vhat_sb[tt]
                    ones_ap = _ap(vt[:].tensor, 64, [[16 * 65, 128], [65, 16]])
                    nc.vector.tensor_copy(ones_ap, onescol[:])
                    for fc in range(2):
                        ps = ps_s.tile([128, 512], F32, name="psv", tag="s")
                        for d in range(8):
                            nc.tensor.matmul(
                                ps[:],
                                xT_sb[d][:, 128 * tt:128 * (tt + 1)],
                                Wv_sb[d][:, 512 * fc:512 * (fc + 1)],
                                start=(d == 0), stop=False,
                            )
                        nc.tensor.matmul(
                            ps[:], ones1[:], bvp_sb[:, 512 * fc:512 * (fc + 1)],
                            start=False, stop=True,
                        )
                        src = _ap(ps[:].tensor, ps[:].offset,
                                  [[512, 128], [64, 8], [1, 64]])
                        dst = _ap(vt[:].tensor, 65 * 8 * fc,
                                  [[16 * 65, 128], [65, 8], [1, 64]])
                        nc.scalar.copy(dst, src)

            # ---- attention-only pools + head loop ----
            with (
                tc.tile_pool(name="expp", bufs=2) as expp,
                tc.tile_pool(name="dtp", bufs=2) as dtp,
                tc.tile_pool(name="sm", bufs=2) as sm,
                tc.tile_pool(name="outtp", bufs=2) as outtp,
                tc.tile_pool(name="denp", bufs=2) as denp,
                tc.tile_pool(name="ps_o", bufs=1, space="PSUM") as ps_o,
                tc.tile_pool(name="ps_dt", bufs=1, space="PSUM") as ps_dt,
            ):
                dt_ps_tile = ps_dt.tile([128, 1024], F32, name="dtps",
                                        tag="dt")

                def emit_scores(h):
                    hp, hh = h // 2, h % 2
                    po = 64 * hh
                    par = h % 2
                    qT = qk_tiles[hp][0]
                    kT = qk_tiles[hp][1]
                    band = band_tiles.pop(h)
                    eA = expp.tile([128, 4096], BF16, name=f"eA{h}", tag="eA")
                    eB = expp.tile([128, 4096], BF16, name=f"eB{h}", tag="eB")
                    pso = ps_o.tile([65, 1024], F32, name="pso", tag="pso")

                    def slot(jt):
                        return (eA, 1024 * jt) if jt < 4 else \
                            (eB, 1024 * (jt - 4))

                    pso_pending = []

                    def flush_pso(upto, final=False):
                        keep = [p for p in pso_pending if p[0] > upto]
                        todo = [p for p in pso_pending if p[0] <= upto]
                        for i, (jt2, a, b) in enumerate(todo):
                            j02 = 128 * jt2
                            t2, o2 = slot(jt2)
                            nc.tensor.matmul(
                                pso[:, a:b],
                                vhat_sb[jt2][:, 65 * h:65 * h + 65],
                                t2[:, o2 + a - j02:o2 + b - j02],
                                start=(jt2 == 0),
                                stop=(final and i == len(todo) - 1),
                                skip_group_check=True)
                        pso_pending.clear()
                        pso_pending.extend(keep)

                    for jt in range(8):
                        j0 = 128 * jt
                        t, o = slot(jt)
                        win = min(144, S - j0)
                        for si, (a, b) in enumerate(SEGS[jt]):
                            psS = ps_s.tile([128, 512], F32, name="psS",
                                            tag="s")
                            first = (si == 0)
                            nc.tensor.matmul(
                                psS[:, 0:b - a],
                                kT[po:po + 64, j0:j0 + 128],
                                qT[po:po + 64, a:b],
                                start=True, stop=not first,
                                skip_group_check=True)
                            if first:
                                c0 = 0
                                bw = b - a
                                nc.tensor.matmul(
                                    psS[:, c0:c0 + min(144, bw)], identB[:],
                                    band[:, 144 * jt:144 * jt + min(144, bw)],
                                    start=False, stop=True,
                                    skip_group_check=True)
                                nc.scalar.activation(
                                    t[:, o:o + bw], psS[:, c0:c0 + bw], EXPF)
                                if jt == 7:
                                    # pad cols 128..144 of the jt7 slot: the
                                    # ewd write reads 144 cols per slot
                                    nc.vector.memset(t[:, o + 128:o + 144],
                                                     0.0)
                            else:
                                nc.scalar.activation(
                                    t[:, o + a - j0:o + b - j0],
                                    psS[:, 0:b - a], EXPF)
                        lo = j0
                        if lo < 512:
                            pso_pending.append((jt, lo, 512))
                            pso_pending.append((jt, 512, 1024))
                        else:
                            pso_pending.append((jt, lo, 1024))
                        flush_pso(jt - 2)
                        if jt == 3:
                            nc.gpsimd.dma_start(
                                out=_ap(ewd[par][:].tensor, 0,
                                        [[144, 128], [EWD_BLK, 4], [1, 144]]),
                                in_=_ap(eA[:].tensor, 0,
                                        [[4096, 128], [1024, 4], [1, 144]]))
                    flush_pso(7, final=True)
                    nc.gpsimd.dma_start(
                        out=_ap(ewd[par][:].tensor, 4 * EWD_BLK,
                                [[144, 128], [EWD_BLK, 4], [1, 144]]),
                        in_=_ap(eB[:].tensor, 0,
                                [[4096, 128], [1024, 4], [1, 144]]))
                    return pso

                def emit_esk_read(h):
                    par = h % 2
                    esk = sm.tile([128, 128], BF16, name=f"esk{h}", tag="esk")
                    nc.sync.dma_start(
                        out=_ap(esk[:].tensor, 0,
                                [[128, 128], [16, 8], [1, 16]]),
                        in_=_ap(ewd[par][:].tensor, 0,
                                [[145, 128], [EWD_BLK, 8], [1, 16]]))
                    return esk

                def emit_dt_back(h, esk):
                    par = h % 2
                    for half in range(2):
                        psE = ps_tr.tile([16, 512], BF16, name="psE",
                                         tag="tr")
                        for q in range(4):
                            jt = 4 * half + q
                            nc.tensor.transpose(
                                psE[:, 128 * q:128 * (q + 1)],
                                esk[:, 16 * jt:16 * (jt + 1)],
                                identB[:])
                        nc.vector.tensor_copy(
                            esT_t[par][:, 512 * half:512 * (half + 1)],
                            psE[:])
                    nc.gpsimd.dma_start(
                        out=_ap(zb1c[par][:].tensor, 0,
                                [[146, 16], [ZB1_BLK, 8], [1, 128]]),
                        in_=_ap(esT_t[par][:].tensor, 0,
                                [[1024, 16], [128, 8], [1, 128]]))
                    dt_all = dtp.tile([16, 8 * 144], BF16, name=f"dt{h}",
                                      tag="dt")
                    nc.sync.dma_start(
                        out=_ap(dt_all[:].tensor, 0,
                                [[8 * 144, 16], [144, 8], [1, 144]]),
                        in_=_ap(zb1c[par][:].tensor, 0,
                                [[145, 16], [ZB1_BLK, 8], [1, 144]]))
                    dt_tiles[h] = dt_all

                def emit_dt_mms(h):
                    # per 128-query tile: one start=True matmul from the own
                    # block's window plus a 16-col start=False overlap matmul
                    # from the previous block (always a subrange, so PSUM
                    # accumulate regions stay uniform)
                    hh = h % 2
                    ph = 64 * hh
                    dt_all = dt_tiles.pop(h)
                    for ct in range(8):
                        q0 = 128 * ct
                        nc.tensor.matmul(
                            dt_ps_tile[ph:ph + 64, q0:q0 + 128], dlv_sb[:],
                            dt_all[:, 144 * ct:144 * ct + 128],
                            start=True, stop=False, skip_group_check=True)
                        if ct >= 1:
                            nc.tensor.matmul(
                                dt_ps_tile[ph:ph + 64, q0:q0 + 16], dlv_sb[:],
                                dt_all[:, 144 * (ct - 1) + 128:
                                       144 * (ct - 1) + 144],
                                start=False, stop=(ct == 7),
                                skip_group_check=True)
                    if hh == 1:
                        hp = h // 2
                        nc.vector.tensor_add(pair_sb[hp][:], pair_sb[hp][:],
                                             dt_ps_tile[:])
                        # inline normalize for this pair
                        den2 = den_tiles.pop(hp)
                        recip2 = sm.tile([2, 1024], F32R, name=f"rc{hp}",
                                         tag="rc")
                        nc.vector.reciprocal(recip2[:], den2[:])
                        for c in range(2):
                            psb = ps_s.tile([128, 512], F32, name="psb",
                                            tag="s")
                            nc.tensor.matmul(
                                psb[:], selm2_sb[:],
                                recip2[:, 512 * c:512 * (c + 1)],
                                start=True, stop=True)
                            nc.vector.tensor_mul(
                                pair_sb[hp][:, 512 * c:512 * (c + 1)],
                                pair_sb[hp][:, 512 * c:512 * (c + 1)],
                                psb[:])

                def emit_evict(h, pso):
                    hp, hh = h // 2, h % 2
                    po = 64 * hh
                    outT = outtp.tile([65, 1024], BF16, name=f"outT{h}",
                                      tag="outT")
                    nc.vector.tensor_copy(outT[:], pso[:])
                    nc.sync.dma_start(out=pair_sb[hp][po:po + 64, :],
                                      in_=outT[0:64, :])
                    if hh == 0:
                        den_tiles[hp] = denp.tile([2, 1024], BF16,
                                                  name=f"den{hp}", tag="den")
                    nc.sync.dma_start(out=den_tiles[hp][hh:hh + 1, :],
                                      in_=outT[64:65, :])

                for h in range(16):
                    hp, hh = h // 2, h % 2
                    if hh == 0:
                        if hp + 2 < 8:
                            emit_wload(hp + 2)
                        if hp + 1 < 8:
                            emit_qkproj(hp + 1)
                            emit_band_stage1(2 * hp + 2)
                            emit_band_stage1(2 * hp + 3)
                    esk_prev = emit_esk_read(h - 1) if h >= 1 else None
                    pso = emit_scores(h)
                    if h + 2 < 16:
                        emit_band_stage2(h + 2)
                    if h >= 1:
                        emit_dt_back(h - 1, esk_prev)
                    if h >= 2:
                        emit_dt_mms(h - 2)
                    emit_evict(h, pso)
                    if hh == 1:
                        qk_tiles.pop(hp, None)

                esk15 = emit_esk_read(15)
                emit_dt_back(15, esk15)
                emit_dt_mms(14)
                emit_dt_mms(15)

            # ---- final projection ----
            with (
                tc.tile_pool(name="wp", bufs=1) as wpp,
                tc.tile_pool(name="ps_p", bufs=2, space="PSUM") as ps_p,
                tc.tile_pool(name="outp", bufs=2) as outp,
            ):
                Wp_sb = []
                for d in range(8):
                    t = wpp.tile([128, D], BF16, name=f"wpt{d}", tag=f"wp{d}")
                    nc.gpsimd.dma_start(out=t[:],
                                        in_=Wp[128 * d:128 * (d + 1), :])
                    Wp_sb.append(t)
                for tt in range(8):
                    ps = ps_p.tile([128, 1024], F32, name="psp", tag="psp")
                    for fc in range(2):
                        for d in range(8):
                            nc.tensor.matmul(
                                ps[:, 512 * fc:512 * (fc + 1)],
                                pair_sb[d][:, 128 * tt:128 * (tt + 1)],
                                Wp_sb[d][:, 512 * fc:512 * (fc + 1)],
                                start=(d == 0), stop=False,
                            )
                        nc.tensor.matmul(
                            ps[:, 512 * fc:512 * (fc + 1)],
                            ones1[:], bp_sb[:, 512 * fc:512 * (fc + 1)],
                            start=False, stop=True,
                        )
                    ot = outp.tile([128, 1024], F32, name="ot", tag="ot")
                    nc.vector.tensor_copy(ot[:], ps[:])
                    nc.sync.dma_start(out=OUT[128 * tt:128 * (tt + 1), :],
                                      in_=ot[:])

    nc.compile()
    return nc


def _host_prep(W_attn, b_attn, W_proj, b_proj, lut_k, lut_v):
    scale = 1.0 / math.sqrt(d_k)
    Wqk = np.concatenate([W_attn[:, :D], W_attn[:, D:2 * D] * scale], axis=1)
    bq = b_attn[:D]
    bk = b_attn[D:2 * D] * scale
    bqk_h = np.stack([np.concatenate([bq, bk])[128 * ft:128 * (ft + 1)]
                      for ft in range(16)], axis=1).astype(np.float32)
    bvp_h = (b_attn[2 * D:3 * D] + np.tile(lut_v[0], N_H)).reshape(1, D)
    dlut_h = np.stack([(lut_k[16 - u] - lut_k[0]) * scale for u in range(16)],
                      axis=1).astype(np.float32)
    dlv_h = np.stack([lut_v[16 - u] - lut_v[0] for u in range(16)],
                     axis=0).astype(ml_dtypes.bfloat16)
    selm2_h = np.zeros((2, 128), np.float32)
    for p in range(128):
        selm2_h[p // 64, p] = 1.0
    blk = np.zeros((128, 161), np.float32)
    cols = np.arange(161)[None, :]
    rows = np.arange(128)[:, None]
    blk[cols < rows] = MASKVAL
    zb2c_h = np.tile(blk.reshape(-1), 8).astype(ml_dtypes.bfloat16)
    zb1c_h = np.zeros(ZB1_SZ, ml_dtypes.bfloat16)
    return {
        "Wqk": np.ascontiguousarray(Wqk).astype(ml_dtypes.bfloat16),
        "Wv": np.ascontiguousarray(W_attn[:, 2 * D:3 * D]).astype(ml_dtypes.bfloat16),
        "Wp": np.ascontiguousarray(W_proj).astype(ml_dtypes.bfloat16),
        "bqk": bqk_h,
        "bvp": np.ascontiguousarray(bvp_h).astype(ml_dtypes.bfloat16),
        "bp": np.ascontiguousarray(
            np.asarray(b_proj).reshape(1, D)).astype(ml_dtypes.bfloat16),
        "dlut": dlut_h.astype(ml_dtypes.bfloat16),
        "dlv": dlv_h,
        "selm2": selm2_h,
        "zb2c0": zb2c_h,
        "zb2c1": zb2c_h.copy(),
        "zb1c0": zb1c_h,
        "zb1c1": zb1c_h.copy(),
    }


def kernel(x, W_attn, b_attn, W_proj, b_proj, lut_k, lut_v):
    x = np.asarray(x, np.float32)
    shared = _host_prep(np.asarray(W_attn, np.float32),
                        np.asarray(b_attn, np.float32),
                        np.asarray(W_proj, np.float32),
                        np.asarray(b_proj, np.float32),
                        np.asarray(lut_k, np.float32),
                        np.asarray(lut_v, np.float32))
    if "nc" not in _CACHE:
        _CACHE["nc"] = build_module()
    nc = _CACHE["nc"]
    in_maps = []
    for b in range(N_CORES):
        m = dict(shared)
        m["xT"] = np.ascontiguousarray(x[b].T).astype(ml_dtypes.bfloat16)
        in_maps.append(m)
    res = run_bass_kernel_spmd(nc, in_maps, list(range(N_CORES)), trace=TRACE)
    _CACHE["last_result"] = res
    out = np.stack([res.results[b]["OUT"] for b in range(N_CORES)], axis=0)
    return out.astype(np.float32)


# revision 37
# speedup vs baseline: 2.2028x; 1.0100x over previous
import sys

sys.path.insert(0, "/opt/trn_rl_repo")

import math

import numpy as np
import ml_dtypes

import concourse.bass as bass
import concourse.mybir as mybir
import concourse.tile as tile
from concourse import bacc
from concourse.bass_utils import run_bass_kernel_spmd
from concourse.masks import make_identity

F32 = mybir.dt.float32
F32R = mybir.dt.float32r
BF16 = mybir.dt.bfloat16
IDENT = mybir.ActivationFunctionType.Identity
EXPF = mybir.ActivationFunctionType.Exp

B, S, D = 8, 1024, 1024
N_H = 16
REL_K = 16
d_k = D // N_H  # 64
N_CORES = 8
MASKVAL = -1e30

# Score segments per key-block jt: absolute query ranges, each one matmul
# (<=512 cols; f32r needs >=256 cols for full rate).  First segment of each
# block carries the band add; jt7 is padded to 256 cols (768..896 computed
# but never exp'd).
SEGS = {
    0: [(0, 512), (512, 1024)],
    1: [(128, 512), (512, 1024)],
    2: [(256, 512), (512, 1024)],
    3: [(384, 768), (768, 1024)],
    4: [(512, 1024)],
    5: [(640, 1024)],
    6: [(768, 1024)],
    7: [(896, 1024)],
}

ZDP_SZ = 17 * 1040
EWD_BLK = 128 * 144
EWD_SZ = 8 * EWD_BLK
ZB2_BLK = 128 * 161
ZB2_SZ = 8 * ZB2_BLK
ZB1_BLK = 16 * 145
ZB1_SZ = 8 * ZB1_BLK

_CACHE = {}
TRACE = False


def _ap(t, offset, dims):
    return bass.AP(tensor=t, offset=offset, ap=[list(d) for d in dims])


def build_module():
    nc = bacc.Bacc("TRN2", detect_race_conditions=False, num_swdge_queues=4)

    xT = nc.dram_tensor("xT", [D, S], BF16, kind="ExternalInput")
    Wqk = nc.dram_tensor("Wqk", [D, 2 * D], BF16, kind="ExternalInput")
    Wv = nc.dram_tensor("Wv", [D, D], BF16, kind="ExternalInput")
    Wp = nc.dram_tensor("Wp", [D, D], BF16, kind="ExternalInput")
    bqk = nc.dram_tensor("bqk", [128, 16], F32, kind="ExternalInput")
    bvp = nc.dram_tensor("bvp", [1, D], BF16, kind="ExternalInput")
    bp = nc.dram_tensor("bp", [1, D], BF16, kind="ExternalInput")
    dlut = nc.dram_tensor("dlut", [d_k, 16], BF16, kind="ExternalInput")
    dlv = nc.dram_tensor("dlv", [16, d_k], BF16, kind="ExternalInput")
    selm2 = nc.dram_tensor("selm2", [2, 128], F32R, kind="ExternalInput")
    zb2c = [nc.dram_tensor(f"zb2c{i}", [ZB2_SZ], BF16, kind="ExternalInput")
            for i in range(2)]
    zb1c = [nc.dram_tensor(f"zb1c{i}", [ZB1_SZ], BF16, kind="ExternalInput")
            for i in range(2)]
    OUT = nc.dram_tensor("OUT", [S, D], F32, kind="ExternalOutput")

    zdp = [nc.dram_tensor(f"zdp{i}", [ZDP_SZ], BF16) for i in range(2)]
    ewd = [nc.dram_tensor(f"ewd{i}", [EWD_SZ], BF16) for i in range(2)]

    with tile.TileContext(nc) as tc, nc.allow_low_precision(
            reason="bf16 attention weights/corrections within tolerance"), \
            tc.tile_pool(name="persist", bufs=1) as pers:
        # ---- xT first: the first qkproj matmuls gate the whole pipeline ----
        xT_sb = []
        for d in range(8):
            t = pers.tile([128, S], BF16, name=f"xTt{d}", tag=f"xT{d}")
            nc.sync.dma_start(out=t[:], in_=xT[128 * d:128 * (d + 1), :])
            xT_sb.append(t)

        # ---- constants (scalar queue; keep SP free for xT) ----
        ident = pers.tile([128, 128], F32)
        make_identity(nc, ident[:])
        identB = pers.tile([128, 128], BF16)
        nc.vector.tensor_copy(identB[:], ident[:])
        dlut_sb = pers.tile([128, 16], BF16)
        nc.scalar.dma_start(out=dlut_sb[0:64, :], in_=dlut[:])
        nc.scalar.dma_start(out=dlut_sb[64:128, :], in_=dlut[:])
        dlv_sb = pers.tile([16, d_k], BF16)
        nc.scalar.dma_start(out=dlv_sb[:], in_=dlv[:])
        selm2_sb = pers.tile([2, 128], F32R)
        nc.scalar.dma_start(out=selm2_sb[:], in_=selm2[:])
        bqk_sb = pers.tile([128, 16], F32)
        nc.scalar.dma_start(out=bqk_sb[:], in_=bqk[:])
        bvp_sb = pers.tile([1, D], BF16)
        nc.scalar.dma_start(out=bvp_sb[:], in_=bvp[:])
        bp_sb = pers.tile([1, D], BF16)
        nc.scalar.dma_start(out=bp_sb[:], in_=bp[:])
        ones1 = pers.tile([1, 128], BF16)
        nc.vector.memset(ones1[:], 1.0)
        onescol = pers.tile([128, 16], BF16)
        nc.vector.memset(onescol[:], 1.0)

        dpT_t = [pers.tile([16, 1040], BF16, name=f"dpT{i}", tag=f"dpT{i}")
                 for i in range(2)]
        for i in range(2):
            nc.vector.memset(dpT_t[i][:, 1024:1040], 0.0)
        dpSh_t = [pers.tile([16, 1040], BF16, name=f"dpSh{i}", tag=f"dpSh{i}")
                  for i in range(2)]
        dpS_t = [pers.tile([128, 128], BF16, name=f"dpS{i}", tag=f"dpS{i}")
                 for i in range(2)]
        esT_t = [pers.tile([16, 1024], BF16, name=f"esT{i}", tag=f"esT{i}")
                 for i in range(2)]

        vhat_sb = [pers.tile([128, 16 * 65], BF16, name=f"vh{jt}",
                             tag=f"vh{jt}") for jt in range(8)]
        pair_sb = [pers.tile([128, S], BF16, name=f"pair{hp}",
                             tag=f"pair{hp}") for hp in range(8)]

        with (
            tc.tile_pool(name="wqkp", bufs=3) as wqkp,
            tc.tile_pool(name="qkp", bufs=3) as qkp,
            tc.tile_pool(name="bandp", bufs=4) as bandp,
            tc.tile_pool(name="ps_s", bufs=4, space="PSUM") as ps_s,
            tc.tile_pool(name="ps_tr", bufs=2, space="PSUM") as ps_tr,
        ):
            wq_tiles = {}
            qk_tiles = {}
            band_tiles = {}
            dt_tiles = {}
            den_tiles = {}

            def emit_wload(hp):
                ws = []
                for sec, ft in ((0, hp), (1, 8 + hp)):
                    w = wqkp.tile([128, 1024], BF16, name=f"wqk{hp}_{sec}",
                                  tag=f"wqk{sec}")
                    src = _ap(Wqk[:].tensor, 128 * ft,
                              [[2 * D, 128], [128 * 2 * D, 8], [1, 128]])
                    dst = _ap(w[:].tensor, 0,
                              [[1024, 128], [128, 8], [1, 128]])
                    nc.gpsimd.dma_start(out=dst, in_=src)
                    ws.append(w)
                wq_tiles[hp] = ws

            def qkproj_fillers(hp):
                ws = wq_tiles.pop(hp)
                pair = [qkp.tile([128, S], BF16, name=f"qk{hp}_0", tag="qk0"),
                        qkp.tile([128, S], BF16, name=f"qk{hp}_1", tag="qk1")]
                qk_tiles[hp] = pair

                def chunk(sec, ft, tch):
                    def emit():
                        ps = ps_s.tile([128, 512], F32, name="psqk", tag="s")
                        for d in range(8):
                            nc.tensor.matmul(
                                ps[:],
                                ws[sec][:, 128 * d:128 * (d + 1)],
                                xT_sb[d][:, 512 * tch:512 * (tch + 1)],
                                start=(d == 0), stop=(d == 7),
                            )
                        nc.scalar.activation(
                            pair[sec][:, 512 * tch:512 * (tch + 1)], ps[:],
                            IDENT, bias=bqk_sb[:, ft:ft + 1], scale=1.0)
                    return emit
                return [chunk(sec, ft, tch) for sec, ft in ((0, hp), (1, 8 + hp))
                        for tch in range(2)]

            def emit_qkproj(hp):
                for f in qkproj_fillers(hp):
                    f()

            def emit_band_stage1(h):
                hp, hh = h // 2, h % 2
                po = 64 * hh
                par = h % 2
                qT = qk_tiles[hp][0]
                for c in range(2):
                    psdp = ps_s.tile([128, 512], F32, name="psdp", tag="s")
                    nc.tensor.matmul(
                        psdp[0:16, :], dlut_sb[po:po + 64, :],
                        qT[po:po + 64, 512 * c:512 * (c + 1)],
                        start=True, stop=True)
                    nc.vector.tensor_copy(
                        dpT_t[par][:, 512 * c:512 * (c + 1)], psdp[0:16, :])
                nc.gpsimd.dma_start(
                    out=_ap(zdp[par][:].tensor, 0, [[1040, 16], [1, 1040]]),
                    in_=dpT_t[par][:])
                nc.sync.dma_start(
                    out=dpSh_t[par][:],
                    in_=_ap(zdp[par][:].tensor, 0, [[1041, 16], [1, 1040]]))

            def emit_band_stage2(h):
                par = h % 2
                psG = ps_tr.tile([128, 128], BF16, name="psG", tag="tr")
                for jt in range(8):
                    j0 = 128 * jt
                    nc.tensor.transpose(
                        psG[:, 16 * jt:16 * (jt + 1)],
                        dpSh_t[par][:, j0:j0 + 128],
                        identB[0:16, 0:16])
                nc.vector.tensor_copy(dpS_t[par][:], psG[:])
                nc.gpsimd.dma_start(
                    out=_ap(zb2c[par][:].tensor, 0,
                            [[162, 128], [ZB2_BLK, 8], [1, 16]]),
                    in_=_ap(dpS_t[par][:].tensor, 0,
                            [[128, 128], [16, 8], [1, 16]]))
                band = bandp.tile([128, 8 * 144], BF16, name=f"band{h}",
                                  tag="band")
                nc.sync.dma_start(
                    out=_ap(band[:].tensor, 0,
                            [[8 * 144, 128], [144, 8], [1, 144]]),
                    in_=_ap(zb2c[par][:].tensor, 0,
                            [[161, 128], [ZB2_BLK, 8], [1, 144]]))
                band_tiles[h] = band

            # ---- bootstrap: qkproj(0) + band stage1 before vproj ----
            emit_wload(0)
            emit_wload(1)
            emit_qkproj(0)
            emit_band_stage1(0)
            emit_band_stage1(1)

            # ---- v projection (PE-dense; hides band bootstrap latency) ----
            with tc.tile_pool(name="wv", bufs=1) as wvp:
                Wv_sb = []
                for d in range(8):
                    t = wvp.tile([128, D], BF16, name=f"wvt{d}", tag=f"wv{d}")
                    nc.gpsimd.dma_start(out=t[:],
                                        in_=Wv[128 * d:128 * (d + 1), :])
                    Wv_sb.append(t)
                for tt in range(8):
                    if tt == 4:
                        emit_band_stage2(0)
                        emit_band_stage2(1)
                    vt = vhat_sb[tt]
                    ones_ap = _ap(vt[:].tensor, 64, [[16 * 65, 128], [65, 16]])
                    nc.vector.tensor_copy(ones_ap, onescol[:])
                    for fc in range(2):
                        ps = ps_s.tile([128, 512], F32, name="psv", tag="s")
                        for d in range(8):
                            nc.tensor.matmul(
                                ps[:],
                                xT_sb[d][:, 128 * tt:128 * (tt + 1)],
                                Wv_sb[d][:, 512 * fc:512 * (fc + 1)],
                                start=(d == 0), stop=False,
                            )
                        nc.tensor.matmul(
                            ps[:], ones1[:], bvp_sb[:, 512 * fc:512 * (fc + 1)],
                            start=False, stop=True,
                        )
                        src = _ap(ps[:].tensor, ps[:].offset,
                                  [[512, 128], [64, 8], [1, 64]])
                        dst = _ap(vt[:].tensor, 65 * 8 * fc,
                                  [[16 * 65, 128], [65, 8], [1, 64]])
                        nc.scalar.copy(dst, src)

            # ---- attention-only pools + head loop ----
            with (
                tc.tile_pool(name="expp", bufs=3) as expp,
                tc.tile_pool(name="dtp", bufs=3) as dtp,
                tc.tile_pool(name="sm", bufs=3) as sm,
                tc.tile_pool(name="outtp", bufs=3) as outtp,
                tc.tile_pool(name="denp", bufs=2) as denp,
                tc.tile_pool(name="ps_o", bufs=1, space="PSUM") as ps_o,
            ):

                def emit_scores(h, fillers=()):
                    fillers = list(fillers)
                    hp, hh = h // 2, h % 2
                    po = 64 * hh
                    par = h % 2
                    qT = qk_tiles[hp][0]
                    kT = qk_tiles[hp][1]
                    band = band_tiles.pop(h)
                    eA = expp.tile([128, 4096], BF16, name=f"eA{h}", tag="eA")
                    eB = expp.tile([128, 4096], BF16, name=f"eB{h}", tag="eB")
                    pso = ps_o.tile([65, 1024], F32, name="pso", tag="pso")

                    def slot(jt):
                        return (eA, 1024 * jt) if jt < 4 else \
                            (eB, 1024 * (jt - 4))

                    pso_pending = []

                    def flush_pso(upto, final=False):
                        keep = [p for p in pso_pending if p[0] > upto]
                        todo = [p for p in pso_pending if p[0] <= upto]
                        for i, (jt2, a, b) in enumerate(todo):
                            j02 = 128 * jt2
                            t2, o2 = slot(jt2)
                            nc.tensor.matmul(
                                pso[:, a:b],
                                vhat_sb[jt2][:, 65 * h:65 * h + 65],
                                t2[:, o2 + a - j02:o2 + b - j02],
                                start=(jt2 == 0),
                                stop=(final and i == len(todo) - 1),
                                skip_group_check=True)
                        pso_pending.clear()
                        pso_pending.extend(keep)

                    for jt in range(8):
                        j0 = 128 * jt
                        t, o = slot(jt)
                        win = min(144, S - j0)
                        for si, (a, b) in enumerate(SEGS[jt]):
                            psS = ps_s.tile([128, 512], F32, name="psS",
                                            tag="s")
                            first = (si == 0)
                            nc.tensor.matmul(
                                psS[:, 0:b - a],
                                kT[po:po + 64, j0:j0 + 128],
                                qT[po:po + 64, a:b],
                                start=True, stop=not first,
                                skip_group_check=True)
                            if first:
                                c0 = 0
                                bw = b - a
                                nc.tensor.matmul(
                                    psS[:, c0:c0 + min(144, bw)], identB[:],
                                    band[:, 144 * jt:144 * jt + min(144, bw)],
                                    start=False, stop=True,
                                    skip_group_check=True)
                                nc.scalar.activation(
                                    t[:, o:o + bw], psS[:, c0:c0 + bw], EXPF)
                                if jt == 7:
                                    # pad cols 128..144 of the jt7 slot: the
                                    # ewd write reads 144 cols per slot
                                    nc.vector.memset(t[:, o + 128:o + 144],
                                                     0.0)
                            else:
                                nc.scalar.activation(
                                    t[:, o + a - j0:o + b - j0],
                                    psS[:, 0:b - a], EXPF)
                        lo = j0
                        if lo < 512:
                            pso_pending.append((jt, lo, 512))
                            pso_pending.append((jt, 512, 1024))
                        else:
                            pso_pending.append((jt, lo, 1024))
                        flush_pso(jt - 2)
                        npop = -(-len(fillers) // (8 - jt)) if jt < 7 else 0
                        for _ in range(min(npop, 2)):
                            fillers.pop(0)()
                        if jt == 3:
                            nc.gpsimd.dma_start(
                                out=_ap(ewd[par][:].tensor, 0,
                                        [[144, 128], [EWD_BLK, 4], [1, 144]]),
                                in_=_ap(eA[:].tensor, 0,
                                        [[4096, 128], [1024, 4], [1, 144]]))
                    flush_pso(7, final=True)
                    for f in fillers:
                        f()
                    nc.gpsimd.dma_start(
                        out=_ap(ewd[par][:].tensor, 4 * EWD_BLK,
                                [[144, 128], [EWD_BLK, 4], [1, 144]]),
                        in_=_ap(eB[:].tensor, 0,
                                [[4096, 128], [1024, 4], [1, 144]]))
                    return pso

                def emit_esk_read(h):
                    par = h % 2
                    esk = sm.tile([128, 128], BF16, name=f"esk{h}", tag="esk")
                    nc.sync.dma_start(
                        out=_ap(esk[:].tensor, 0,
                                [[128, 128], [16, 8], [1, 16]]),
                        in_=_ap(ewd[par][:].tensor, 0,
                                [[145, 128], [EWD_BLK, 8], [1, 16]]))
                    return esk

                def dt_back_half(h, esk, half):
                    par = h % 2
                    psE = ps_tr.tile([16, 512], BF16, name="psE", tag="tr")
                    for q in range(4):
                        jt = 4 * half + q
                        nc.tensor.transpose(
                            psE[:, 128 * q:128 * (q + 1)],
                            esk[:, 16 * jt:16 * (jt + 1)],
                            identB[:])
                    nc.vector.tensor_copy(
                        esT_t[par][:, 512 * half:512 * (half + 1)], psE[:])
                    if half == 1:
                        nc.gpsimd.dma_start(
                            out=_ap(zb1c[par][:].tensor, 0,
                                    [[146, 16], [ZB1_BLK, 8], [1, 128]]),
                            in_=_ap(esT_t[par][:].tensor, 0,
                                    [[1024, 16], [128, 8], [1, 128]]))
                        dt_all = dtp.tile([16, 8 * 144], BF16, name=f"dt{h}",
                                          tag="dt")
                        nc.sync.dma_start(
                            out=_ap(dt_all[:].tensor, 0,
                                    [[8 * 144, 16], [144, 8], [1, 144]]),
                            in_=_ap(zb1c[par][:].tensor, 0,
                                    [[145, 16], [ZB1_BLK, 8], [1, 144]]))
                        dt_tiles[h] = dt_all

                def emit_dt_back(h, esk):
                    dt_back_half(h, esk, 0)
                    dt_back_half(h, esk, 1)

                def dt_mm_quad(h, blo):
                    # key-block DT matmuls into a small psum tile + DVE adds
                    # straight into pair_sb rows (overlaps become plain adds)
                    hp, hh = h // 2, h % 2
                    ph = 64 * hh
                    dt_all = dt_tiles[h]
                    for jt in range(blo, blo + 4):
                        j0 = 128 * jt
                        win = min(144, S - j0)
                        psdt = ps_tr.tile([64, 144], F32, name="psdt",
                                          tag="tr")
                        nc.tensor.matmul(
                            psdt[:, 0:win], dlv_sb[:],
                            dt_all[:, 144 * jt:144 * jt + win],
                            start=True, stop=True)
                        nc.vector.tensor_add(
                            pair_sb[hp][ph:ph + 64, j0:j0 + win],
                            pair_sb[hp][ph:ph + 64, j0:j0 + win],
                            psdt[:, 0:win])
                    if blo == 4:
                        dt_tiles.pop(h)

                def normalize_pair(hp):
                    den2 = den_tiles.pop(hp)
                    recip2 = sm.tile([2, 1024], F32R, name=f"rc{hp}",
                                     tag="rc")
                    nc.vector.reciprocal(recip2[:], den2[:])
                    for c in range(2):
                        psb = ps_s.tile([128, 512], F32, name="psb", tag="s")
                        nc.tensor.matmul(
                            psb[:], selm2_sb[:],
                            recip2[:, 512 * c:512 * (c + 1)],
                            start=True, stop=True)
                        nc.vector.tensor_mul(
                            pair_sb[hp][:, 512 * c:512 * (c + 1)],
                            pair_sb[hp][:, 512 * c:512 * (c + 1)],
                            psb[:])

                def emit_dt_mms(h):
                    dt_mm_quad(h, 0)
                    dt_mm_quad(h, 4)
                    if h % 2 == 1:
                        normalize_pair(h // 2)

                def emit_evict(h, pso):
                    hp, hh = h // 2, h % 2
                    po = 64 * hh
                    outT = outtp.tile([65, 1024], BF16, name=f"outT{h}",
                                      tag="outT")
                    nc.vector.tensor_copy(outT[:], pso[:])
                    nc.sync.dma_start(out=pair_sb[hp][po:po + 64, :],
                                      in_=outT[0:64, :])
                    if hh == 0:
                        den_tiles[hp] = denp.tile([2, 1024], BF16,
                                                  name=f"den{hp}", tag="den")
                    nc.sync.dma_start(out=den_tiles[hp][hh:hh + 1, :],
                                      in_=outT[64:65, :])

                for h in range(16):
                    hp, hh = h // 2, h % 2
                    if hh == 0:
                        if hp + 2 < 8:
                            emit_wload(hp + 2)
                        if hp + 1 < 8:
                            emit_qkproj(hp + 1)
                            emit_band_stage1(2 * hp + 2)
                            emit_band_stage1(2 * hp + 3)
                    esk_prev = emit_esk_read(h - 1) if h >= 1 else None
                    pso = emit_scores(h)
                    if h + 2 < 16:
                        emit_band_stage2(h + 2)
                    if h >= 1:
                        emit_dt_back(h - 1, esk_prev)
                    if h >= 2:
                        emit_dt_mms(h - 2)
                    emit_evict(h, pso)
                    if hh == 1:
                        qk_tiles.pop(hp, None)

                esk15 = emit_esk_read(15)
                emit_dt_back(15, esk15)
                emit_dt_mms(14)
                emit_dt_mms(15)

            # ---- final projection ----
            with (
                tc.tile_pool(name="wp", bufs=1) as wpp,
                tc.tile_pool(name="outp", bufs=2) as outp,
            ):
                Wp_sb = []
                for d in range(8):
                    t = wpp.tile([128, D], BF16, name=f"wpt{d}", tag=f"wp{d}")
                    nc.gpsimd.dma_start(out=t[:],
                                        in_=Wp[128 * d:128 * (d + 1), :])
                    Wp_sb.append(t)
                for tt in range(8):
                    ot = outp.tile([128, 1024], F32, name="ot", tag="ot")
                    for fc in range(2):
                        ps = ps_s.tile([128, 512], F32, name="psp", tag="s")
                        for d in range(8):
                            nc.tensor.matmul(
                                ps[:],
                                pair_sb[d][:, 128 * tt:128 * (tt + 1)],
                                Wp_sb[d][:, 512 * fc:512 * (fc + 1)],
                                start=(d == 0), stop=False,
                            )
                        nc.tensor.matmul(
                            ps[:], ones1[:], bp_sb[:, 512 * fc:512 * (fc + 1)],
                            start=False, stop=True,
                        )
                        nc.vector.tensor_copy(
                            ot[:, 512 * fc:512 * (fc + 1)], ps[:])
                    nc.sync.dma_start(out=OUT[128 * tt:128 * (tt + 1), :],
                                      in_=ot[:])

    nc.compile()
    return nc


def _host_prep(W_attn, b_attn, W_proj, b_proj, lut_k, lut_v):
    scale = 1.0 / math.sqrt(d_k)
    Wqk = np.concatenate([W_attn[:, :D], W_attn[:, D:2 * D] * scale], axis=1)
    bq = b_attn[:D]
    bk = b_attn[D:2 * D] * scale
    bqk_h = np.stack([np.concatenate([bq, bk])[128 * ft:128 * (ft + 1)]
                      for ft in range(16)], axis=1).astype(np.float32)
    bvp_h = (b_attn[2 * D:3 * D] + np.tile(lut_v[0], N_H)).reshape(1, D)
    dlut_h = np.stack([(lut_k[16 - u] - lut_k[0]) * scale for u in range(16)],
                      axis=1).astype(np.float32)
    dlv_h = np.stack([lut_v[16 - u] - lut_v[0] for u in range(16)],
                     axis=0).astype(ml_dtypes.bfloat16)
    selm2_h = np.zeros((2, 128), np.float32)
    for p in range(128):
        selm2_h[p // 64, p] = 1.0
    blk = np.zeros((128, 161), np.float32)
    cols = np.arange(161)[None, :]
    rows = np.arange(128)[:, None]
    blk[cols < rows] = MASKVAL
    zb2c_h = np.tile(blk.reshape(-1), 8).astype(ml_dtypes.bfloat16)
    zb1c_h = np.zeros(ZB1_SZ, ml_dtypes.bfloat16)
    return {
        "Wqk": np.ascontiguousarray(Wqk).astype(ml_dtypes.bfloat16),
        "Wv": np.ascontiguousarray(W_attn[:, 2 * D:3 * D]).astype(ml_dtypes.bfloat16),
        "Wp": np.ascontiguousarray(W_proj).astype(ml_dtypes.bfloat16),
        "bqk": bqk_h,
        "bvp": np.ascontiguousarray(bvp_h).astype(ml_dtypes.bfloat16),
        "bp": np.ascontiguousarray(
            np.asarray(b_proj).reshape(1, D)).astype(ml_dtypes.bfloat16),
        "dlut": dlut_h.astype(ml_dtypes.bfloat16),
        "dlv": dlv_h,
        "selm2": selm2_h,
        "zb2c0": zb2c_h,
        "zb2c1": zb2c_h.copy(),
        "zb1c0": zb1c_h,
        "zb1c1": zb1c_h.copy(),
    }


def kernel(x, W_attn, b_attn, W_proj, b_proj, lut_k, lut_v):
    x = np.asarray(x, np.float32)
    shared = _host_prep(np.asarray(W_attn, np.float32),
                        np.asarray(b_attn, np.float32),
                        np.asarray(W_proj, np.float32),
                        np.asarray(b_proj, np.float32),
                        np.asarray(lut_k, np.float32),
                        np.asarray(lut_v, np.float32))
    if "nc" not in _CACHE:
        _CACHE["nc"] = build_module()
    nc = _CACHE["nc"]
    in_maps = []
    for b in range(N_CORES):
        m = dict(shared)
        m["xT"] = np.ascontiguousarray(x[b].T).astype(ml_dtypes.bfloat16)
        in_maps.append(m)
    res = run_bass_kernel_spmd(nc, in_maps, list(range(N_CORES)), trace=TRACE)
    _CACHE["last_result"] = res
    out = np.stack([res.results[b]["OUT"] for b in range(N_CORES)], axis=0)
    return out.astype(np.float32)
